# revision 22
# baseline (speedup 1.0000x reference)
"""Trainium2 Bass kernel for CodePredictorAttention (B=2, Q=2048, HID=2048,
HQ=16, HKV=4, D=128, causal, qk-rmsnorm + neox rope, GQA).

Sharding (8 cores): data-parallel over batch (2) x tensor-parallel over head
groups (4). Core c handles batch c//4 and q-heads [4g, 4g+4) with kv-head g,
g = c%4. o_proj is row-parallel; the 4 partial outputs per batch are summed
on the host.

Per-core pipeline (all matmuls in float32r: full PE speed, ~12-bit mantissa):
  1. qkv projection  out[tok, feat] = x^T-tiles.T @ w-tiles   (feat = 4q+k+v)
  2. rms-norm scale via DVE (sumsq + rsqrt Newton), applied during PSUM
     eviction (ACT copy with per-partition scale); neox rope on DVE;
     q/k transposed to [D, tok] via PE transposes.
  3. attention in S^T layout: S^T[k,q] = kT.T @ qT (+ causal mask tiles via
     identity matmul), E = exp(S^T * scale) on ACT, O^T[D,q] = V.T @ E and
     colsums = ones.T @ E accumulated on PE; normalize O^T = O^T * (1/sums)
     on DVE.
  4. o_proj out[tok, hid] = O^T-tiles.T @ wo^T-tiles, DMA to DRAM.
"""
import os
import numpy as np
from contextlib import ExitStack

import concourse.bass as bass
import concourse.tile as tile
from concourse import bacc, mybir
from concourse.bass_utils import run_bass_kernel_spmd

B, Q, HID = 2, 2048, 2048
HQ, HKV, D = 16, 4, 128
NQH = HQ // HKV          # q heads per core = 4
EPS = 1e-6
THETA = 1000000.0
SCALE = float(D) ** -0.5
MASK_NEG = -30000.0
P = 128
TOK_T = Q // P           # 16 token tiles
KT = HID // P            # 16 hid contraction tiles
QM = 4                   # q-macro tiles of 512
QMW = Q // QM            # 512
F32 = mybir.dt.float32
F32R = mybir.dt.float32r
I32 = mybir.dt.int32
AF = mybir.ActivationFunctionType
OP = mybir.AluOpType

RSQRT_MAGIC = 0x5F3759DF

last_exec_time_ns = None   # set when BASS_TRACE=1


def _emit(ctx, tc, io, apply_qw, apply_kw):
    nc = tc.nc

    const = ctx.enter_context(tc.tile_pool(name="const", bufs=1))
    xpool = ctx.enter_context(tc.tile_pool(name="xp", bufs=3))
    qkvsb = ctx.enter_context(tc.tile_pool(name="qkvsb", bufs=2))
    rsq = ctx.enter_context(tc.tile_pool(name="rsq", bufs=5))
    big = ctx.enter_context(tc.tile_pool(name="big", bufs=1))
    blk = ctx.enter_context(tc.tile_pool(name="blk", bufs=3))
    blko = ctx.enter_context(tc.tile_pool(name="blko", bufs=2))
    epool = ctx.enter_context(tc.tile_pool(name="ep", bufs=3))
    opool = ctx.enter_context(tc.tile_pool(name="op", bufs=2))
    recp = ctx.enter_context(tc.tile_pool(name="recp", bufs=2))
    scrp = ctx.enter_context(tc.tile_pool(name="scrp", bufs=2))
    psum = ctx.enter_context(tc.tile_pool(name="ps", bufs=6, space="PSUM"))
    psum_kv = ctx.enter_context(tc.tile_pool(name="pskv", bufs=2, space="PSUM"))

    # ---- earliest x tiles first: the very first matmuls need them ----
    early_x = {}
    for t0 in range(2):
        ex = xpool.tile([P, KT, P], F32R, tag="x", name=f"x{t0}")
        for kc in range(0, KT, 8):
            nc.sync.dma_start(ex[:, kc:kc + 8, :], io["xt"][:, t0, kc:kc + 8, :])
        early_x[t0] = ex

    # ---- resident constants / weights ----
    w_sb = const.tile([P, KT, 512 + 2 * P], F32R, tag="wbig")  # qkv w [p, kt, f]
    # early k-slices split in halves across queues; first matmuls start early
    FW = 512 + 2 * P
    for k in range(KT):
        if k < 4:
            nc.sync.dma_start(w_sb[:, k, 0:FW // 2],
                              io["wt"][k * P:(k + 1) * P, 0:FW // 2])
            nc.sync.dma_start(w_sb[:, k, FW // 2:],
                              io["wt"][k * P:(k + 1) * P, FW // 2:])
        else:
            nc.sync.dma_start(w_sb[:, k, :],
                              io["wt"][k * P:(k + 1) * P, :])
    cos_sb = const.tile([P, TOK_T, D // 2], F32)
    nc.sync.dma_start(cos_sb[:], io["cos"][:])
    sin_sb = const.tile([P, TOK_T, D // 2], F32)
    nc.sync.dma_start(sin_sb[:], io["sin"][:])
    mask_sb = const.tile([P, NQH, QMW], F32R)
    nc.sync.dma_start(mask_sb[:], io["masks"][:])
    ident_sb = const.tile([P, P], F32R)
    nc.sync.dma_start(ident_sb[:], io["ident"][:])
    ones_sb = const.tile([P, P], F32R)
    nc.sync.dma_start(ones_sb[:], io["ones"][:])
    if apply_qw:
        wqrep_sb = const.tile([P, NQH * P], F32)
        nc.sync.dma_start(wqrep_sb[:], io["wqrep"][:])
    if apply_kw:
        wkrep_sb = const.tile([P, P], F32)
        nc.sync.dma_start(wkrep_sb[:], io["wkrep"][:])
    magic_sb = const.tile([P, NQH + 1], I32)
    nc.vector.memset(magic_sb[:], RSQRT_MAGIC)

    # ---- resident activations ----
    kT_sb = big.tile([P, Q], F32R)           # [D, tok]
    v_sb = big.tile([P, TOK_T, D], F32R)     # [tok%128, t, D]

    def rsqrt_dve(out_ap, in_ap, n):
        """out = in^-1/2 elementwise on DVE: bit-trick seed + 2 Newton steps."""
        y = rsq.tile([P, n], F32, tag="rs_y")
        sh = rsq.tile([P, n], I32, tag="rs_sh")
        nc.vector.tensor_scalar(sh[:], in_ap.bitcast(I32), 1, None,
                                op0=OP.arith_shift_right)
        nc.vector.tensor_sub(y[:].bitcast(I32), magic_sb[:, :n], sh[:])
        for it in range(2):
            a = rsq.tile([P, n], F32, tag="rs_a")
            c = rsq.tile([P, n], F32, tag="rs_c")
            nc.vector.tensor_mul(a[:], y[:], y[:])
            nc.vector.tensor_mul(a[:], a[:], in_ap)
            nc.vector.tensor_scalar(c[:], a[:], -0.5, 1.5, op0=OP.mult, op1=OP.add)
            if it == 0:
                yn = rsq.tile([P, n], F32, tag="rs_y2")
                nc.vector.tensor_mul(yn[:], y[:], c[:])
                y = yn
            else:
                nc.vector.tensor_mul(out_ap, y[:], c[:])

    # o_proj weights resident alongside qkv weights; DMA emitted later (at
    # the start of block j=1) so it does not crowd startup bandwidth
    wo_sb = const.tile([P, NQH, HID], F32R)

    NH = NQH + 1
    d2 = D // 2
    sq_scale = float(D) ** -0.5

    qkn_tiles = {}
    x_tiles = dict(early_x)

    def prefetch_x(t):
        x_sb = xpool.tile([P, KT, P], F32R, tag="x", name=f"x{t}")
        for kc in range(0, KT, 8):
            nc.sync.dma_start(x_sb[:, kc:kc + 8, :],
                              io["xt"][:, t, kc:kc + 8, :])
        x_tiles[t] = x_sb

    def qkv_block(t):
        """qkv matmuls + rmsnorm + rope for token tile t (qk_n stashed for
        the separately-emitted tp_block)."""
        if t not in x_tiles:
            prefetch_x(t)
        x_sb = x_tiles.pop(t)
        qps = psum.tile([P, NQH * P], F32, tag="a", name=f"qps{t}")
        kvps = psum_kv.tile([P, 2 * P], F32, tag="kv", name=f"kvps{t}")
        for k in range(KT):
            nc.tensor.matmul(qps[:], x_sb[:, k, :], w_sb[:, k, 0:NQH * P],
                             start=(k == 0), stop=(k == KT - 1))
        for k in range(KT):
            nc.tensor.matmul(kvps[:], x_sb[:, k, :], w_sb[:, k, NQH * P:],
                             start=(k == 0), stop=(k == KT - 1))

        # evict PSUM fast; k stacked behind the 4 q heads (5 lanes)
        qk = qkvsb.tile([P, NH * P], F32, tag="qk", name=f"qk{t}")
        nc.scalar.copy(v_sb[:, t, :], kvps[:, P:2 * P])
        nc.scalar.copy(qk[:, NQH * P:], kvps[:, 0:P])
        nc.scalar.copy(qk[:, 0:NQH * P], qps[:])

        # mean-square per lane: ACT Square (in every table set) + accum_out
        msq = rsq.tile([P, NH], F32, tag="msq", name=f"msq{t}")
        for h in range(NH):
            scr = scrp.tile([P, P], F32, tag="scr", name=f"scr{t}_{h}")
            nc.scalar.activation(scr[:], qk[:, h * P:(h + 1) * P], AF.Square,
                                 scale=sq_scale, accum_out=msq[:, h:h + 1])
        msqe = rsq.tile([P, NH], F32, tag="msqe", name=f"msqe{t}")
        nc.vector.tensor_scalar(msqe[:], msq[:], EPS, None, op0=OP.add)
        rstd = rsq.tile([P, NH], F32, tag="rstd", name=f"rstd{t}")
        rsqrt_dve(rstd[:], msqe[:], NH)

        # per-lane rstd scale in place on DVE
        for h in range(NH):
            nc.vector.tensor_scalar(qk[:, h * P:(h + 1) * P],
                                    qk[:, h * P:(h + 1) * P],
                                    rstd[:, h:h + 1], None, op0=OP.mult)
        if apply_qw:
            nc.vector.tensor_mul(qk[:, 0:NQH * P], qk[:, 0:NQH * P],
                                 wqrep_sb[:])
        if apply_kw:
            nc.vector.tensor_mul(qk[:, NQH * P:], qk[:, NQH * P:],
                                 wkrep_sb[:])

        # neox rope fused across the 5 lanes; subtract/add reuse qk_n in place
        cosb = cos_sb[:, t:t + 1, :].to_broadcast([P, NH, d2])
        sinb = sin_sb[:, t:t + 1, :].to_broadcast([P, NH, d2])
        qv = qk[:].rearrange("p (h d) -> p h d", h=NH)
        qk_n = qkvsb.tile([P, NH * P], F32R, tag="qkn", name=f"qkn{t}")
        qnv = qk_n[:].rearrange("p (h d) -> p h d", h=NH)
        t1 = qkvsb.tile([P, NH * d2], F32, tag="t1", name=f"t1_{t}")
        t1v = t1[:].rearrange("p (h d) -> p h d", h=NH)
        nc.vector.tensor_mul(qnv[:, :, 0:d2], qv[:, :, d2:D], sinb)
        nc.vector.tensor_mul(t1v, qv[:, :, 0:d2], cosb)
        nc.vector.tensor_sub(qnv[:, :, 0:d2], t1v, qnv[:, :, 0:d2])
        nc.vector.tensor_mul(qnv[:, :, d2:D], qv[:, :, 0:d2], sinb)
        nc.vector.tensor_mul(t1v, qv[:, :, d2:D], cosb)
        nc.vector.tensor_add(qnv[:, :, d2:D], t1v, qnv[:, :, d2:D])
        qkn_tiles[t] = qk_n

    def tp_block(t, qTb):
        """transpose the 5 rope'd lanes into [D, tok] stores (DVE evicts)"""
        tb = (t % 4) * P
        qk_n = qkn_tiles.pop(t)
        for h in range(NH):
            tp = psum.tile([P, P], F32R, tag="a", name=f"tp{t}_{h}")
            nc.tensor.transpose(tp[:], qk_n[:, h * P:(h + 1) * P], ident_sb[:])
            if h < NQH:
                nc.vector.tensor_copy(qTb[:, h, tb:tb + P], tp[:])
            else:
                nc.vector.tensor_copy(kT_sb[:, t * P:(t + 1) * P], tp[:])

    def attn_block(h, j, qTb, otb):
        """causal attention for head h, q-macro j (S^T layout), software-
        pipelined so S(i+1) runs on PE while ACT computes exp(i)."""
        nk = 4 * j + 4
        ops_ = psum.tile([P, QMW], F32, tag="a", name=f"ops{h}_{j}")
        sums = psum_kv.tile([P, QMW], F32, tag="kv", name=f"sums{h}_{j}")

        def s_mm(i):
            diag = i >= 4 * j
            sps = psum.tile([P, QMW], F32, tag="a", name=f"sps{h}_{j}_{i}")
            nc.tensor.matmul(sps[:], kT_sb[:, i * P:(i + 1) * P],
                             qTb[:, h, :], start=True, stop=not diag)
            if diag:
                nc.tensor.matmul(sps[:], ident_sb[:],
                                 mask_sb[:, i - 4 * j, :],
                                 start=False, stop=True)
            return sps

        sps = s_mm(0)
        for i in range(nk):
            e = epool.tile([P, QMW], F32R, tag="e", name=f"e{h}_{j}_{i}")
            nc.scalar.activation(e[:], sps[:], AF.Exp, scale=SCALE)
            if i + 1 < nk:
                sps = s_mm(i + 1)
            nc.tensor.matmul(ops_[:], v_sb[:, i, :], e[:],
                             start=(i == 0), stop=(i == nk - 1))
            nc.tensor.matmul(sums[:], ones_sb[:], e[:],
                             start=(i == 0), stop=(i == nk - 1))
        rec = recp.tile([P, QMW], F32, tag="rec", name=f"rec{h}_{j}")
        nc.vector.reciprocal_approx_fast(out=rec[:], in_=sums[:])
        nc.vector.tensor_mul(otb[:, h, :], ops_[:], rec[:])

    def oproj_block(t, otb):
        tb = (t % 4) * P
        for nh in range(NQH):
            pps = psum.tile([P, QMW], F32, tag="a", name=f"pps{t}_{nh}")
            for kf in range(NQH):
                nc.tensor.matmul(pps[:], otb[:, kf, tb:tb + P],
                                 wo_sb[:, kf, nh * QMW:(nh + 1) * QMW],
                                 start=(kf == 0), stop=(kf == NQH - 1))
            o_t = opool.tile([P, QMW], F32, tag="oo", name=f"ot{t}_{nh}")
            if nh % 2 == 0:
                nc.vector.tensor_copy(o_t[:], pps[:])
            else:
                nc.scalar.copy(o_t[:], pps[:])
            nc.sync.dma_start(
                io["out"][t * P:(t + 1) * P, nh * QMW:(nh + 1) * QMW], o_t[:])

    # ======= software-pipelined schedule =======
    # Block j's qkv/norm work is interleaved (in each engine's static order)
    # with block j-1's attention + o_proj so the PE never waits on the serial
    # ACT->DVE norm chain.
    for t0 in range(2, 4):
        prefetch_x(t0)
    qTbs, otbs = {}, {}
    LAG = 2   # attention for block j runs while qkv of block j+LAG executes
    for slot in range(QM + LAG):
        j = slot            # qkv block index
        ja = slot - LAG     # attention/oproj block index
        if j == 2:
            for kf in range(NQH):
                nc.sync.dma_start(wo_sb[:, kf, :],
                                  io["wot"][kf * P:(kf + 1) * P, :])
        if j < QM:
            qTbs[j] = blk.tile([P, NQH, QMW], F32R, tag="qtb", name=f"qTb{j}")
            otbs[j] = blko.tile([P, NQH, QMW], F32R, tag="otb", name=f"otb{j}")
        for step in range(4):
            t = 4 * j + step
            if j < QM:
                qkv_block(t)
            if ja >= 0:
                attn_block(step, ja, qTbs[ja], otbs[ja])
            if j < QM:
                tp_block(t, qTbs[j])
        if ja >= 0:
            for t2 in range(4 * ja, 4 * ja + 4):
                oproj_block(t2, otbs[ja])
            del qTbs[ja], otbs[ja]


_cache = {}


def _build(apply_qw, apply_kw):
    key = (apply_qw, apply_kw)
    if key in _cache:
        return _cache[key]
    nc = bacc.Bacc("TRN2", target_bir_lowering=False, debug=False)
    io = {
        "xt": nc.dram_tensor("xt", (P, TOK_T, KT, P), F32R, kind="ExternalInput")[:],
        "wt": nc.dram_tensor("wt", (HID, 512 + 2 * P), F32R, kind="ExternalInput")[:],
        "wot": nc.dram_tensor("wot", (NQH * P, HID), F32R, kind="ExternalInput")[:],
        "cos": nc.dram_tensor("cos", (P, TOK_T, D // 2), F32, kind="ExternalInput")[:],
        "sin": nc.dram_tensor("sin", (P, TOK_T, D // 2), F32, kind="ExternalInput")[:],
        "masks": nc.dram_tensor("masks", (P, NQH, QMW), F32R, kind="ExternalInput")[:],
        "ident": nc.dram_tensor("ident", (P, P), F32R, kind="ExternalInput")[:],
        "ones": nc.dram_tensor("ones", (P, P), F32R, kind="ExternalInput")[:],
        "out": nc.dram_tensor("out", (Q, HID), F32, kind="ExternalOutput")[:],
    }
    if apply_qw:
        io["wqrep"] = nc.dram_tensor("wqrep", (P, NQH * P), F32,
                                     kind="ExternalInput")[:]
    if apply_kw:
        io["wkrep"] = nc.dram_tensor("wkrep", (P, P), F32,
                                     kind="ExternalInput")[:]
    with tile.TileContext(nc) as tc:
        with ExitStack() as ctx:
            _emit(ctx, tc, io, apply_qw, apply_kw)
    nc.compile()
    _cache[key] = nc
    return nc


def kernel(positions, hidden_states, k_cache, v_cache, wqkv, wo, q_norm_w,
           k_norm_w, seq_len):
    global last_exec_time_ns
    positions = np.asarray(positions)
    hidden_states = np.asarray(hidden_states, dtype=np.float32)
    wqkv = np.asarray(wqkv, dtype=np.float32)
    wo = np.asarray(wo, dtype=np.float32)
    q_norm_w = np.asarray(q_norm_w, dtype=np.float32)
    k_norm_w = np.asarray(k_norm_w, dtype=np.float32)
    if int(np.asarray(seq_len)) != Q:
        raise NotImplementedError("kernel compiled for seq_len == qlen == 2048")

    apply_qw = not np.all(q_norm_w == 1.0)
    apply_kw = not np.all(k_norm_w == 1.0)
    nc = _build(apply_qw, apply_kw)

    # rope tables per batch (mirrors reference fp32 arithmetic)
    inv_freq = 1.0 / (np.float32(THETA) **
                      (np.arange(0, D, 2, dtype=np.float32) / np.float32(D)))
    # causal mask tiles for the 4 diagonal offsets
    p_idx = np.arange(P, dtype=np.int64)[:, None]
    f_idx = np.arange(QMW, dtype=np.int64)[None, :]
    masks = np.zeros((NQH, P, QMW), dtype=np.float32)
    for r in range(NQH):
        masks[r] = np.where(f_idx >= p_idx + r * P, 0.0, MASK_NEG)
    masksr = np.ascontiguousarray(masks.transpose(1, 0, 2))  # [p, r, f]
    ident = np.eye(P, dtype=np.float32)
    ones = np.ones((P, P), dtype=np.float32)

    in_maps = []
    for c in range(8):
        b, g = c // 4, c % 4
        # pre-tiled x^T: xr[p, t, kt, m] = hidden[b][t*128+m, kt*128+p]
        xt = np.ascontiguousarray(
            hidden_states[b].T.reshape(KT, P, TOK_T, P).transpose(1, 2, 0, 3))
        wq = wqkv[512 * g:512 * (g + 1)]
        wk = wqkv[HQ * D + P * g: HQ * D + P * (g + 1)]
        wv = wqkv[HQ * D + HKV * D + P * g: HQ * D + HKV * D + P * (g + 1)]
        wt = np.ascontiguousarray(np.concatenate([wq, wk, wv], axis=0).T)
        wot = np.ascontiguousarray(wo[:, 512 * g:512 * (g + 1)].T)
        freqs = positions[b].astype(np.float32)[:, None] * inv_freq[None, :]
        cosf = np.cos(freqs).astype(np.float32)
        sinf = np.sin(freqs).astype(np.float32)
        # pre-tiled [p, t, d] layouts for 4KB DMA descriptors
        cosr = np.ascontiguousarray(
            cosf.reshape(TOK_T, P, D // 2).transpose(1, 0, 2))
        sinr = np.ascontiguousarray(
            sinf.reshape(TOK_T, P, D // 2).transpose(1, 0, 2))
        m = {
            "xt": xt, "wt": wt, "wot": wot,
            "cos": cosr, "sin": sinr,
            "masks": masksr, "ident": ident, "ones": ones,
        }
        if apply_qw:
            m["wqrep"] = np.broadcast_to(
                np.tile(q_norm_w, NQH)[None, :], (P, NQH * P)).copy()
        if apply_kw:
            m["wkrep"] = np.broadcast_to(k_norm_w[None, :], (P, P)).copy()
        in_maps.append(m)

    trace = bool(os.environ.get("BASS_TRACE"))
    res = run_bass_kernel_spmd(nc, in_maps, core_ids=list(range(8)),
                               trace=trace)
    last_exec_time_ns = res.exec_time_ns

    out = np.empty((B, Q, HID), dtype=np.float32)
    for b in range(B):
        acc = res.results[4 * b]["out"].astype(np.float32).copy()
        for g in range(1, 4):
            acc += res.results[4 * b + g]["out"]
        out[b] = acc
    return out


# revision 23
# speedup vs baseline: 1.0796x; 1.0796x over previous
"""Trainium2 Bass kernel for CodePredictorAttention (B=2, Q=2048, HID=2048,
HQ=16, HKV=4, D=128, causal, qk-rmsnorm + neox rope, GQA).

Sharding (8 cores): data-parallel over batch (2) x tensor-parallel over head
groups (4). Core c handles batch c//4 and q-heads [4g, 4g+4) with kv-head g,
g = c%4. o_proj is row-parallel; the 4 partial outputs per batch are summed
on the host.

Per-core pipeline (all matmuls in float32r: full PE speed, ~12-bit mantissa):
  1. qkv projection  out[tok, feat] = x^T-tiles.T @ w-tiles   (feat = 4q+k+v)
  2. rms-norm scale via DVE (sumsq + rsqrt Newton), applied during PSUM
     eviction (ACT copy with per-partition scale); neox rope on DVE;
     q/k transposed to [D, tok] via PE transposes.
  3. attention in S^T layout: S^T[k,q] = kT.T @ qT (+ causal mask tiles via
     identity matmul), E = exp(S^T * scale) on ACT, O^T[D,q] = V.T @ E and
     colsums = ones.T @ E accumulated on PE; normalize O^T = O^T * (1/sums)
     on DVE.
  4. o_proj out[tok, hid] = O^T-tiles.T @ wo^T-tiles, DMA to DRAM.
"""
import os
import numpy as np
from contextlib import ExitStack

import concourse.bass as bass
import concourse.tile as tile
from concourse import bacc, mybir
from concourse.bass_utils import run_bass_kernel_spmd

B, Q, HID = 2, 2048, 2048
HQ, HKV, D = 16, 4, 128
NQH = HQ // HKV          # q heads per core = 4
EPS = 1e-6
THETA = 1000000.0
SCALE = float(D) ** -0.5
MASK_NEG = -30000.0
P = 128
TOK_T = Q // P           # 16 token tiles
KT = HID // P            # 16 hid contraction tiles
QM = 4                   # q-macro tiles of 512
QMW = Q // QM            # 512
F32 = mybir.dt.float32
F32R = mybir.dt.float32r
I32 = mybir.dt.int32
AF = mybir.ActivationFunctionType
OP = mybir.AluOpType

RSQRT_MAGIC = 0x5F3759DF

last_exec_time_ns = None   # set when BASS_TRACE=1


def _emit(ctx, tc, io, apply_qw, apply_kw):
    nc = tc.nc

    const = ctx.enter_context(tc.tile_pool(name="const", bufs=1))
    xpool = ctx.enter_context(tc.tile_pool(name="xp", bufs=4))
    qkvsb = ctx.enter_context(tc.tile_pool(name="qkvsb", bufs=2))
    rsq = ctx.enter_context(tc.tile_pool(name="rsq", bufs=5))
    big = ctx.enter_context(tc.tile_pool(name="big", bufs=1))
    blk = ctx.enter_context(tc.tile_pool(name="blk", bufs=2))
    blko = ctx.enter_context(tc.tile_pool(name="blko", bufs=2))
    epool = ctx.enter_context(tc.tile_pool(name="ep", bufs=3))
    opool = ctx.enter_context(tc.tile_pool(name="op", bufs=2))
    recp = ctx.enter_context(tc.tile_pool(name="recp", bufs=2))
    scrp = ctx.enter_context(tc.tile_pool(name="scrp", bufs=2))
    psum = ctx.enter_context(tc.tile_pool(name="ps", bufs=6, space="PSUM"))
    psum_kv = ctx.enter_context(tc.tile_pool(name="pskv", bufs=2, space="PSUM"))

    # ---- earliest x tiles first: the very first matmuls need them ----
    early_x = {}
    for t0 in range(2):
        ex = xpool.tile([P, KT, P], F32R, tag="x", name=f"x{t0}")
        for kc in range(0, KT, 8):
            nc.sync.dma_start(ex[:, kc:kc + 8, :], io["xt"][:, t0, kc:kc + 8, :])
        early_x[t0] = ex

    # ---- resident constants / weights ----
    w_sb = const.tile([P, KT, 512 + 2 * P], F32R, tag="wbig")  # qkv w [p, kt, f]
    # early k-slices split in halves across queues; first matmuls start early
    FW = 512 + 2 * P
    for k in range(KT):
        if k < 4:
            nc.sync.dma_start(w_sb[:, k, 0:FW // 2],
                              io["wt"][k * P:(k + 1) * P, 0:FW // 2])
            nc.sync.dma_start(w_sb[:, k, FW // 2:],
                              io["wt"][k * P:(k + 1) * P, FW // 2:])
        else:
            nc.sync.dma_start(w_sb[:, k, :],
                              io["wt"][k * P:(k + 1) * P, :])
    cos_sb = const.tile([P, TOK_T, D // 2], F32)
    nc.sync.dma_start(cos_sb[:], io["cos"][:])
    sin_sb = const.tile([P, TOK_T, D // 2], F32)
    nc.sync.dma_start(sin_sb[:], io["sin"][:])
    mask_sb = const.tile([P, NQH, QMW], F32R)
    nc.sync.dma_start(mask_sb[:], io["masks"][:])
    ident_sb = const.tile([P, P], F32R)
    nc.sync.dma_start(ident_sb[:], io["ident"][:])
    ones_sb = const.tile([P, P], F32R)
    nc.sync.dma_start(ones_sb[:], io["ones"][:])
    if apply_qw:
        wqrep_sb = const.tile([P, NQH * P], F32)
        nc.sync.dma_start(wqrep_sb[:], io["wqrep"][:])
    if apply_kw:
        wkrep_sb = const.tile([P, P], F32)
        nc.sync.dma_start(wkrep_sb[:], io["wkrep"][:])
    magic_sb = const.tile([P, NQH + 1], I32)
    nc.vector.memset(magic_sb[:], RSQRT_MAGIC)

    # ---- resident activations ----
    kT_sb = big.tile([P, Q], F32R)           # [D, tok]
    v_sb = big.tile([P, TOK_T, D], F32R)     # [tok%128, t, D]

    def rsqrt_dve(out_ap, in_ap, n):
        """out = in^-1/2 elementwise on DVE: bit-trick seed + 2 Newton steps."""
        y = rsq.tile([P, n], F32, tag="rs_y")
        sh = rsq.tile([P, n], I32, tag="rs_sh")
        nc.vector.tensor_scalar(sh[:], in_ap.bitcast(I32), 1, None,
                                op0=OP.arith_shift_right)
        nc.vector.tensor_sub(y[:].bitcast(I32), magic_sb[:, :n], sh[:])
        for it in range(2):
            a = rsq.tile([P, n], F32, tag="rs_a")
            c = rsq.tile([P, n], F32, tag="rs_c")
            nc.vector.tensor_mul(a[:], y[:], y[:])
            nc.vector.tensor_mul(a[:], a[:], in_ap)
            nc.vector.tensor_scalar(c[:], a[:], -0.5, 1.5, op0=OP.mult, op1=OP.add)
            if it == 0:
                yn = rsq.tile([P, n], F32, tag="rs_y2")
                nc.vector.tensor_mul(yn[:], y[:], c[:])
                y = yn
            else:
                nc.vector.tensor_mul(out_ap, y[:], c[:])

    # o_proj weights resident alongside qkv weights; DMA emitted later (at
    # the start of block j=1) so it does not crowd startup bandwidth
    wo_sb = const.tile([P, NQH, HID], F32R)

    NH = NQH + 1
    d2 = D // 2
    sq_scale = float(D) ** -0.5

    qkn_tiles = {}
    x_tiles = dict(early_x)

    def prefetch_x(t):
        x_sb = xpool.tile([P, KT, P], F32R, tag="x", name=f"x{t}")
        for kc in range(0, KT, 8):
            nc.sync.dma_start(x_sb[:, kc:kc + 8, :],
                              io["xt"][:, t, kc:kc + 8, :])
        x_tiles[t] = x_sb

    def qkv_block(t):
        """qkv matmuls + rmsnorm + rope for token tile t (qk_n stashed for
        the separately-emitted tp_block)."""
        if t not in x_tiles:
            prefetch_x(t)
        x_sb = x_tiles.pop(t)
        qps = psum.tile([P, NQH * P], F32, tag="a", name=f"qps{t}")
        kvps = psum_kv.tile([P, 2 * P], F32, tag="kv", name=f"kvps{t}")
        for k in range(KT):
            nc.tensor.matmul(qps[:], x_sb[:, k, :], w_sb[:, k, 0:NQH * P],
                             start=(k == 0), stop=(k == KT - 1))
        for k in range(KT):
            nc.tensor.matmul(kvps[:], x_sb[:, k, :], w_sb[:, k, NQH * P:],
                             start=(k == 0), stop=(k == KT - 1))

        # evict PSUM fast; k stacked behind the 4 q heads (5 lanes)
        qk = qkvsb.tile([P, NH * P], F32, tag="qk", name=f"qk{t}")
        nc.scalar.copy(v_sb[:, t, :], kvps[:, P:2 * P])
        nc.scalar.copy(qk[:, NQH * P:], kvps[:, 0:P])
        nc.scalar.copy(qk[:, 0:NQH * P], qps[:])

        # mean-square per lane: ACT Square (in every table set) + accum_out
        msq = rsq.tile([P, NH], F32, tag="msq", name=f"msq{t}")
        for h in range(NH):
            scr = scrp.tile([P, P], F32, tag="scr", name=f"scr{t}_{h}")
            nc.scalar.activation(scr[:], qk[:, h * P:(h + 1) * P], AF.Square,
                                 scale=sq_scale, accum_out=msq[:, h:h + 1])
        msqe = rsq.tile([P, NH], F32, tag="msqe", name=f"msqe{t}")
        nc.vector.tensor_scalar(msqe[:], msq[:], EPS, None, op0=OP.add)
        rstd = rsq.tile([P, NH], F32, tag="rstd", name=f"rstd{t}")
        rsqrt_dve(rstd[:], msqe[:], NH)

        # per-lane rstd scale in place on DVE
        for h in range(NH):
            nc.vector.tensor_scalar(qk[:, h * P:(h + 1) * P],
                                    qk[:, h * P:(h + 1) * P],
                                    rstd[:, h:h + 1], None, op0=OP.mult)
        if apply_qw:
            nc.vector.tensor_mul(qk[:, 0:NQH * P], qk[:, 0:NQH * P],
                                 wqrep_sb[:])
        if apply_kw:
            nc.vector.tensor_mul(qk[:, NQH * P:], qk[:, NQH * P:],
                                 wkrep_sb[:])

        # neox rope fused across the 5 lanes; subtract/add reuse qk_n in place
        cosb = cos_sb[:, t:t + 1, :].to_broadcast([P, NH, d2])
        sinb = sin_sb[:, t:t + 1, :].to_broadcast([P, NH, d2])
        qv = qk[:].rearrange("p (h d) -> p h d", h=NH)
        qk_n = qkvsb.tile([P, NH * P], F32R, tag="qkn", name=f"qkn{t}")
        qnv = qk_n[:].rearrange("p (h d) -> p h d", h=NH)
        t1 = qkvsb.tile([P, NH * d2], F32, tag="t1", name=f"t1_{t}")
        t1v = t1[:].rearrange("p (h d) -> p h d", h=NH)
        nc.vector.tensor_mul(qnv[:, :, 0:d2], qv[:, :, d2:D], sinb)
        nc.vector.tensor_mul(t1v, qv[:, :, 0:d2], cosb)
        nc.vector.tensor_sub(qnv[:, :, 0:d2], t1v, qnv[:, :, 0:d2])
        nc.vector.tensor_mul(qnv[:, :, d2:D], qv[:, :, 0:d2], sinb)
        nc.vector.tensor_mul(t1v, qv[:, :, d2:D], cosb)
        nc.vector.tensor_add(qnv[:, :, d2:D], t1v, qnv[:, :, d2:D])
        qkn_tiles[t] = qk_n

    def tp_block(t, qTb):
        """transpose the 5 rope'd lanes into [D, tok] stores (DVE evicts)"""
        tb = (t % 4) * P
        qk_n = qkn_tiles.pop(t)
        for h in range(NH):
            tp = psum.tile([P, P], F32R, tag="a", name=f"tp{t}_{h}")
            nc.tensor.transpose(tp[:], qk_n[:, h * P:(h + 1) * P], ident_sb[:])
            if h < NQH:
                nc.vector.tensor_copy(qTb[:, h, tb:tb + P], tp[:])
            else:
                nc.vector.tensor_copy(kT_sb[:, t * P:(t + 1) * P], tp[:])

    def attn_block(h, j, qTb, otb):
        """causal attention for head h, q-macro j (S^T layout), software-
        pipelined so S(i+1) runs on PE while ACT computes exp(i)."""
        nk = 4 * j + 4
        ops_ = psum.tile([P, QMW], F32, tag="a", name=f"ops{h}_{j}")
        sums = psum_kv.tile([P, QMW], F32, tag="kv", name=f"sums{h}_{j}")

        def s_mm(i):
            diag = i >= 4 * j
            sps = psum.tile([P, QMW], F32, tag="a", name=f"sps{h}_{j}_{i}")
            nc.tensor.matmul(sps[:], kT_sb[:, i * P:(i + 1) * P],
                             qTb[:, h, :], start=True, stop=not diag)
            if diag:
                nc.tensor.matmul(sps[:], ident_sb[:],
                                 mask_sb[:, i - 4 * j, :],
                                 start=False, stop=True)
            return sps

        sps = s_mm(0)
        for i in range(nk):
            e = epool.tile([P, QMW], F32R, tag="e", name=f"e{h}_{j}_{i}")
            nc.scalar.activation(e[:], sps[:], AF.Exp, scale=SCALE)
            if i + 1 < nk:
                sps = s_mm(i + 1)
            nc.tensor.matmul(ops_[:], v_sb[:, i, :], e[:],
                             start=(i == 0), stop=(i == nk - 1))
            nc.tensor.matmul(sums[:], ones_sb[:], e[:],
                             start=(i == 0), stop=(i == nk - 1))
        rec = recp.tile([P, QMW], F32, tag="rec", name=f"rec{h}_{j}")
        nc.vector.reciprocal_approx_fast(out=rec[:], in_=sums[:])
        nc.vector.tensor_mul(otb[:, h, :], ops_[:], rec[:])

    def oproj_block(t, otb):
        tb = (t % 4) * P
        for nh in range(NQH):
            pps = psum.tile([P, QMW], F32, tag="a", name=f"pps{t}_{nh}")
            for kf in range(NQH):
                nc.tensor.matmul(pps[:], otb[:, kf, tb:tb + P],
                                 wo_sb[:, kf, nh * QMW:(nh + 1) * QMW],
                                 start=(kf == 0), stop=(kf == NQH - 1))
            o_t = opool.tile([P, QMW], F32, tag="oo", name=f"ot{t}_{nh}")
            if nh % 2 == 0:
                nc.vector.tensor_copy(o_t[:], pps[:])
            else:
                nc.scalar.copy(o_t[:], pps[:])
            nc.sync.dma_start(
                io["out"][t * P:(t + 1) * P, nh * QMW:(nh + 1) * QMW], o_t[:])

    # ======= software-pipelined schedule =======
    # Block j's qkv/norm work is interleaved (in each engine's static order)
    # with block j-1's attention + o_proj so the PE never waits on the serial
    # ACT->DVE norm chain.
    for t0 in range(2, 4):
        prefetch_x(t0)
    qTbs, otbs = {}, {}
    LAG = 1   # attention for block j runs while qkv of block j+LAG executes
    for slot in range(QM + LAG):
        j = slot            # qkv block index
        ja = slot - LAG     # attention/oproj block index
        if j == 1:
            for kf in range(NQH):
                nc.sync.dma_start(wo_sb[:, kf, :],
                                  io["wot"][kf * P:(kf + 1) * P, :])
        if j < QM:
            qTbs[j] = blk.tile([P, NQH, QMW], F32R, tag="qtb", name=f"qTb{j}")
            otbs[j] = blko.tile([P, NQH, QMW], F32R, tag="otb", name=f"otb{j}")
        for step in range(4):
            t = 4 * j + step
            if j < QM:
                qkv_block(t)
            if ja >= 0:
                attn_block(step, ja, qTbs[ja], otbs[ja])
            if j < QM:
                tp_block(t, qTbs[j])
        if ja >= 0:
            for t2 in range(4 * ja, 4 * ja + 4):
                oproj_block(t2, otbs[ja])
            del qTbs[ja], otbs[ja]


_cache = {}


def _build(apply_qw, apply_kw):
    key = (apply_qw, apply_kw)
    if key in _cache:
        return _cache[key]
    nc = bacc.Bacc("TRN2", target_bir_lowering=False, debug=False)
    io = {
        "xt": nc.dram_tensor("xt", (P, TOK_T, KT, P), F32R, kind="ExternalInput")[:],
        "wt": nc.dram_tensor("wt", (HID, 512 + 2 * P), F32R, kind="ExternalInput")[:],
        "wot": nc.dram_tensor("wot", (NQH * P, HID), F32R, kind="ExternalInput")[:],
        "cos": nc.dram_tensor("cos", (P, TOK_T, D // 2), F32, kind="ExternalInput")[:],
        "sin": nc.dram_tensor("sin", (P, TOK_T, D // 2), F32, kind="ExternalInput")[:],
        "masks": nc.dram_tensor("masks", (P, NQH, QMW), F32R, kind="ExternalInput")[:],
        "ident": nc.dram_tensor("ident", (P, P), F32R, kind="ExternalInput")[:],
        "ones": nc.dram_tensor("ones", (P, P), F32R, kind="ExternalInput")[:],
        "out": nc.dram_tensor("out", (Q, HID), F32, kind="ExternalOutput")[:],
    }
    if apply_qw:
        io["wqrep"] = nc.dram_tensor("wqrep", (P, NQH * P), F32,
                                     kind="ExternalInput")[:]
    if apply_kw:
        io["wkrep"] = nc.dram_tensor("wkrep", (P, P), F32,
                                     kind="ExternalInput")[:]
    with tile.TileContext(nc) as tc:
        with ExitStack() as ctx:
            _emit(ctx, tc, io, apply_qw, apply_kw)
    nc.compile()
    _cache[key] = nc
    return nc


def kernel(positions, hidden_states, k_cache, v_cache, wqkv, wo, q_norm_w,
           k_norm_w, seq_len):
    global last_exec_time_ns
    positions = np.asarray(positions)
    hidden_states = np.asarray(hidden_states, dtype=np.float32)
    wqkv = np.asarray(wqkv, dtype=np.float32)
    wo = np.asarray(wo, dtype=np.float32)
    q_norm_w = np.asarray(q_norm_w, dtype=np.float32)
    k_norm_w = np.asarray(k_norm_w, dtype=np.float32)
    if int(np.asarray(seq_len)) != Q:
        raise NotImplementedError("kernel compiled for seq_len == qlen == 2048")

    apply_qw = not np.all(q_norm_w == 1.0)
    apply_kw = not np.all(k_norm_w == 1.0)
    nc = _build(apply_qw, apply_kw)

    # rope tables per batch (mirrors reference fp32 arithmetic)
    inv_freq = 1.0 / (np.float32(THETA) **
                      (np.arange(0, D, 2, dtype=np.float32) / np.float32(D)))
    # causal mask tiles for the 4 diagonal offsets
    p_idx = np.arange(P, dtype=np.int64)[:, None]
    f_idx = np.arange(QMW, dtype=np.int64)[None, :]
    masks = np.zeros((NQH, P, QMW), dtype=np.float32)
    for r in range(NQH):
        masks[r] = np.where(f_idx >= p_idx + r * P, 0.0, MASK_NEG)
    masksr = np.ascontiguousarray(masks.transpose(1, 0, 2))  # [p, r, f]
    ident = np.eye(P, dtype=np.float32)
    ones = np.ones((P, P), dtype=np.float32)

    in_maps = []
    for c in range(8):
        b, g = c // 4, c % 4
        # pre-tiled x^T: xr[p, t, kt, m] = hidden[b][t*128+m, kt*128+p]
        xt = np.ascontiguousarray(
            hidden_states[b].T.reshape(KT, P, TOK_T, P).transpose(1, 2, 0, 3))
        wq = wqkv[512 * g:512 * (g + 1)]
        wk = wqkv[HQ * D + P * g: HQ * D + P * (g + 1)]
        wv = wqkv[HQ * D + HKV * D + P * g: HQ * D + HKV * D + P * (g + 1)]
        wt = np.ascontiguousarray(np.concatenate([wq, wk, wv], axis=0).T)
        wot = np.ascontiguousarray(wo[:, 512 * g:512 * (g + 1)].T)
        freqs = positions[b].astype(np.float32)[:, None] * inv_freq[None, :]
        cosf = np.cos(freqs).astype(np.float32)
        sinf = np.sin(freqs).astype(np.float32)
        # pre-tiled [p, t, d] layouts for 4KB DMA descriptors
        cosr = np.ascontiguousarray(
            cosf.reshape(TOK_T, P, D // 2).transpose(1, 0, 2))
        sinr = np.ascontiguousarray(
            sinf.reshape(TOK_T, P, D // 2).transpose(1, 0, 2))
        m = {
            "xt": xt, "wt": wt, "wot": wot,
            "cos": cosr, "sin": sinr,
            "masks": masksr, "ident": ident, "ones": ones,
        }
        if apply_qw:
            m["wqrep"] = np.broadcast_to(
                np.tile(q_norm_w, NQH)[None, :], (P, NQH * P)).copy()
        if apply_kw:
            m["wkrep"] = np.broadcast_to(k_norm_w[None, :], (P, P)).copy()
        in_maps.append(m)

    trace = bool(os.environ.get("BASS_TRACE"))
    res = run_bass_kernel_spmd(nc, in_maps, core_ids=list(range(8)),
                               trace=trace)
    last_exec_time_ns = res.exec_time_ns

    out = np.empty((B, Q, HID), dtype=np.float32)
    for b in range(B):
        acc = res.results[4 * b]["out"].astype(np.float32).copy()
        for g in range(1, 4):
            acc += res.results[4 * b + g]["out"]
        out[b] = acc
    return out


# revision 24
# speedup vs baseline: 1.0910x; 1.0106x over previous
"""Trainium2 Bass kernel for CodePredictorAttention (B=2, Q=2048, HID=2048,
HQ=16, HKV=4, D=128, causal, qk-rmsnorm + neox rope, GQA).

Sharding (8 cores): data-parallel over batch (2) x tensor-parallel over head
groups (4). Core c handles batch c//4 and q-heads [4g, 4g+4) with kv-head g,
g = c%4. o_proj is row-parallel; the 4 partial outputs per batch are summed
on the host.

Per-core pipeline (all matmuls in float32r: full PE speed, ~12-bit mantissa):
  1. qkv projection  out[tok, feat] = x^T-tiles.T @ w-tiles   (feat = 4q+k+v)
  2. rms-norm scale via DVE (sumsq + rsqrt Newton), applied during PSUM
     eviction (ACT copy with per-partition scale); neox rope on DVE;
     q/k transposed to [D, tok] via PE transposes.
  3. attention in S^T layout: S^T[k,q] = kT.T @ qT (+ causal mask tiles via
     identity matmul), E = exp(S^T * scale) on ACT, O^T[D,q] = V.T @ E and
     colsums = ones.T @ E accumulated on PE; normalize O^T = O^T * (1/sums)
     on DVE.
  4. o_proj out[tok, hid] = O^T-tiles.T @ wo^T-tiles, DMA to DRAM.
"""
import os
import numpy as np
from contextlib import ExitStack

import concourse.bass as bass
import concourse.tile as tile
from concourse import bacc, mybir
from concourse.bass_utils import run_bass_kernel_spmd

B, Q, HID = 2, 2048, 2048
HQ, HKV, D = 16, 4, 128
NQH = HQ // HKV          # q heads per core = 4
EPS = 1e-6
THETA = 1000000.0
SCALE = float(D) ** -0.5
MASK_NEG = -30000.0
P = 128
TOK_T = Q // P           # 16 token tiles
KT = HID // P            # 16 hid contraction tiles
QM = 4                   # q-macro tiles of 512
QMW = Q // QM            # 512
F32 = mybir.dt.float32
F32R = mybir.dt.float32r
I32 = mybir.dt.int32
AF = mybir.ActivationFunctionType
OP = mybir.AluOpType

RSQRT_MAGIC = 0x5F3759DF

last_exec_time_ns = None   # set when BASS_TRACE=1


def _emit(ctx, tc, io, apply_qw, apply_kw):
    nc = tc.nc

    const = ctx.enter_context(tc.tile_pool(name="const", bufs=1))
    xpool = ctx.enter_context(tc.tile_pool(name="xp", bufs=4))
    qkvsb = ctx.enter_context(tc.tile_pool(name="qkvsb", bufs=2))
    rsq = ctx.enter_context(tc.tile_pool(name="rsq", bufs=5))
    big = ctx.enter_context(tc.tile_pool(name="big", bufs=1))
    blk = ctx.enter_context(tc.tile_pool(name="blk", bufs=2))
    blko = ctx.enter_context(tc.tile_pool(name="blko", bufs=2))
    epool = ctx.enter_context(tc.tile_pool(name="ep", bufs=3))
    opool = ctx.enter_context(tc.tile_pool(name="op", bufs=2))
    recp = ctx.enter_context(tc.tile_pool(name="recp", bufs=2))
    scrp = ctx.enter_context(tc.tile_pool(name="scrp", bufs=2))
    psum = ctx.enter_context(tc.tile_pool(name="ps", bufs=6, space="PSUM"))
    psum_kv = ctx.enter_context(tc.tile_pool(name="pskv", bufs=2, space="PSUM"))

    # ---- earliest x tiles first: the very first matmuls need them ----
    early_x = {}
    for t0 in range(2):
        ex = xpool.tile([P, KT, P], F32R, tag="x", name=f"x{t0}")
        for kc in range(0, KT, 8):
            nc.sync.dma_start(ex[:, kc:kc + 8, :], io["xt"][:, t0, kc:kc + 8, :])
        early_x[t0] = ex

    # ---- resident constants / weights ----
    w_sb = const.tile([P, KT, 512 + 2 * P], F32R, tag="wbig")  # qkv w [p, kt, f]
    # early k-slices split in halves across queues; first matmuls start early
    FW = 512 + 2 * P
    for k in range(KT):
        if k < 4:
            nc.sync.dma_start(w_sb[:, k, 0:FW // 2],
                              io["wt"][k * P:(k + 1) * P, 0:FW // 2])
            nc.sync.dma_start(w_sb[:, k, FW // 2:],
                              io["wt"][k * P:(k + 1) * P, FW // 2:])
        else:
            nc.sync.dma_start(w_sb[:, k, :],
                              io["wt"][k * P:(k + 1) * P, :])
    cos_sb = const.tile([P, TOK_T, D // 2], F32)
    nc.sync.dma_start(cos_sb[:], io["cos"][:])
    sin_sb = const.tile([P, TOK_T, D // 2], F32)
    nc.sync.dma_start(sin_sb[:], io["sin"][:])
    mask_sb = const.tile([P, NQH, QMW], F32R)
    nc.sync.dma_start(mask_sb[:], io["masks"][:])
    ident_sb = const.tile([P, P], F32R)
    nc.sync.dma_start(ident_sb[:], io["ident"][:])
    ones_sb = const.tile([P, P], F32R)
    nc.sync.dma_start(ones_sb[:], io["ones"][:])
    if apply_qw:
        wqrep_sb = const.tile([P, NQH * P], F32)
        nc.sync.dma_start(wqrep_sb[:], io["wqrep"][:])
    if apply_kw:
        wkrep_sb = const.tile([P, P], F32)
        nc.sync.dma_start(wkrep_sb[:], io["wkrep"][:])
    magic_sb = const.tile([P, NQH + 1], I32)
    nc.vector.memset(magic_sb[:], RSQRT_MAGIC)

    # ---- resident activations ----
    kT_sb = big.tile([P, Q], F32R)           # [D, tok]
    v_sb = big.tile([P, TOK_T, D], F32R)     # [tok%128, t, D]

    def rsqrt_dve(out_ap, in_ap, n):
        """out = in^-1/2 elementwise on DVE: bit-trick seed + 2 Newton steps."""
        y = rsq.tile([P, n], F32, tag="rs_y")
        sh = rsq.tile([P, n], I32, tag="rs_sh")
        nc.vector.tensor_scalar(sh[:], in_ap.bitcast(I32), 1, None,
                                op0=OP.arith_shift_right)
        nc.vector.tensor_sub(y[:].bitcast(I32), magic_sb[:, :n], sh[:])
        for it in range(2):
            a = rsq.tile([P, n], F32, tag="rs_a")
            c = rsq.tile([P, n], F32, tag="rs_c")
            nc.vector.tensor_mul(a[:], y[:], y[:])
            nc.vector.tensor_mul(a[:], a[:], in_ap)
            nc.vector.tensor_scalar(c[:], a[:], -0.5, 1.5, op0=OP.mult, op1=OP.add)
            if it == 0:
                yn = rsq.tile([P, n], F32, tag="rs_y2")
                nc.vector.tensor_mul(yn[:], y[:], c[:])
                y = yn
            else:
                nc.vector.tensor_mul(out_ap, y[:], c[:])

    # o_proj weights resident alongside qkv weights; DMA emitted later (at
    # the start of block j=1) so it does not crowd startup bandwidth
    wo_sb = const.tile([P, NQH, HID], F32R)

    NH = NQH + 1
    d2 = D // 2
    sq_scale = float(D) ** -0.5

    qkn_tiles = {}
    x_tiles = dict(early_x)

    def prefetch_x(t):
        x_sb = xpool.tile([P, KT, P], F32R, tag="x", name=f"x{t}")
        for kc in range(0, KT, 8):
            nc.sync.dma_start(x_sb[:, kc:kc + 8, :],
                              io["xt"][:, t, kc:kc + 8, :])
        x_tiles[t] = x_sb

    def qkv_block(t):
        """qkv matmuls + rmsnorm + rope for token tile t (qk_n stashed for
        the separately-emitted tp_block)."""
        if t not in x_tiles:
            prefetch_x(t)
        x_sb = x_tiles.pop(t)
        qps = psum.tile([P, NQH * P], F32, tag="a", name=f"qps{t}")
        kvps = psum_kv.tile([P, 2 * P], F32, tag="kv", name=f"kvps{t}")
        for k in range(KT):
            nc.tensor.matmul(qps[:], x_sb[:, k, :], w_sb[:, k, 0:NQH * P],
                             start=(k == 0), stop=(k == KT - 1))
        for k in range(KT):
            nc.tensor.matmul(kvps[:], x_sb[:, k, :], w_sb[:, k, NQH * P:],
                             start=(k == 0), stop=(k == KT - 1))

        # evict PSUM fast; k stacked behind the 4 q heads (5 lanes)
        qk = qkvsb.tile([P, NH * P], F32, tag="qk", name=f"qk{t}")
        nc.scalar.copy(v_sb[:, t, :], kvps[:, P:2 * P])
        nc.scalar.copy(qk[:, NQH * P:], kvps[:, 0:P])
        nc.scalar.copy(qk[:, 0:NQH * P], qps[:])

        # mean-square per lane: ACT Square (in every table set) + accum_out
        msq = rsq.tile([P, NH], F32, tag="msq", name=f"msq{t}")
        for h in range(NH):
            scr = scrp.tile([P, P], F32, tag="scr", name=f"scr{t}_{h}")
            nc.scalar.activation(scr[:], qk[:, h * P:(h + 1) * P], AF.Square,
                                 scale=sq_scale, accum_out=msq[:, h:h + 1])
        msqe = rsq.tile([P, NH], F32, tag="msqe", name=f"msqe{t}")
        nc.vector.tensor_scalar(msqe[:], msq[:], EPS, None, op0=OP.add)
        rstd = rsq.tile([P, NH], F32, tag="rstd", name=f"rstd{t}")
        rsqrt_dve(rstd[:], msqe[:], NH)

        # per-lane rstd scale in place on DVE
        for h in range(NH):
            nc.vector.tensor_scalar(qk[:, h * P:(h + 1) * P],
                                    qk[:, h * P:(h + 1) * P],
                                    rstd[:, h:h + 1], None, op0=OP.mult)
        if apply_qw:
            nc.vector.tensor_mul(qk[:, 0:NQH * P], qk[:, 0:NQH * P],
                                 wqrep_sb[:])
        if apply_kw:
            nc.vector.tensor_mul(qk[:, NQH * P:], qk[:, NQH * P:],
                                 wkrep_sb[:])

        # neox rope fused across the 5 lanes; subtract/add reuse qk_n in place
        cosb = cos_sb[:, t:t + 1, :].to_broadcast([P, NH, d2])
        sinb = sin_sb[:, t:t + 1, :].to_broadcast([P, NH, d2])
        qv = qk[:].rearrange("p (h d) -> p h d", h=NH)
        qk_n = qkvsb.tile([P, NH * P], F32R, tag="qkn", name=f"qkn{t}")
        qnv = qk_n[:].rearrange("p (h d) -> p h d", h=NH)
        t1 = qkvsb.tile([P, NH * d2], F32, tag="t1", name=f"t1_{t}")
        t1v = t1[:].rearrange("p (h d) -> p h d", h=NH)
        nc.vector.tensor_mul(qnv[:, :, 0:d2], qv[:, :, d2:D], sinb)
        nc.vector.tensor_mul(t1v, qv[:, :, 0:d2], cosb)
        nc.vector.tensor_sub(qnv[:, :, 0:d2], t1v, qnv[:, :, 0:d2])
        nc.vector.tensor_mul(qnv[:, :, d2:D], qv[:, :, 0:d2], sinb)
        nc.vector.tensor_mul(t1v, qv[:, :, d2:D], cosb)
        nc.vector.tensor_add(qnv[:, :, d2:D], t1v, qnv[:, :, d2:D])
        qkn_tiles[t] = qk_n

    def tp_block(t, qTb):
        """transpose the 5 rope'd lanes into [D, tok] stores (DVE evicts)"""
        tb = (t % 4) * P
        qk_n = qkn_tiles.pop(t)
        for h in range(NH):
            tp = psum.tile([P, P], F32R, tag="a", name=f"tp{t}_{h}")
            nc.tensor.transpose(tp[:], qk_n[:, h * P:(h + 1) * P], ident_sb[:])
            if h < NQH:
                nc.vector.tensor_copy(qTb[:, h, tb:tb + P], tp[:])
            else:
                nc.vector.tensor_copy(kT_sb[:, t * P:(t + 1) * P], tp[:])

    def attn_block(h, j, qTb, otb):
        """causal attention for head h, q-macro j (S^T layout), software-
        pipelined so S(i+1) runs on PE while ACT computes exp(i)."""
        nk = 4 * j + 4
        ops_ = psum.tile([P, QMW], F32, tag="a", name=f"ops{h}_{j}")
        sums = psum_kv.tile([P, QMW], F32, tag="kv", name=f"sums{h}_{j}")

        def s_off(i):
            # diagonal tile at offset r: columns < 128*r are fully masked
            return max(0, (i - 4 * j)) * P

        def s_mm(i):
            diag = i >= 4 * j
            off = s_off(i)
            sps = psum.tile([P, QMW], F32, tag="a", name=f"sps{h}_{j}_{i}")
            nc.tensor.matmul(sps[:, off:], kT_sb[:, i * P:(i + 1) * P],
                             qTb[:, h, off:], start=True, stop=not diag)
            if diag:
                nc.tensor.matmul(sps[:, off:], ident_sb[:],
                                 mask_sb[:, i - 4 * j, off:],
                                 start=False, stop=True)
            return sps

        sps = s_mm(0)
        for i in range(nk):
            off = s_off(i)
            e = epool.tile([P, QMW], F32R, tag="e", name=f"e{h}_{j}_{i}")
            nc.scalar.activation(e[:, off:], sps[:, off:], AF.Exp, scale=SCALE)
            if i + 1 < nk:
                sps = s_mm(i + 1)
            nc.tensor.matmul(ops_[:, off:], v_sb[:, i, :], e[:, off:],
                             start=(i == 0), stop=(i == nk - 1))
            nc.tensor.matmul(sums[:, off:], ones_sb[:], e[:, off:],
                             start=(i == 0), stop=(i == nk - 1))
        rec = recp.tile([P, QMW], F32, tag="rec", name=f"rec{h}_{j}")
        nc.vector.reciprocal_approx_fast(out=rec[:], in_=sums[:])
        nc.vector.tensor_mul(otb[:, h, :], ops_[:], rec[:])

    def oproj_block(t, otb):
        tb = (t % 4) * P
        for nh in range(NQH):
            pps = psum.tile([P, QMW], F32, tag="a", name=f"pps{t}_{nh}")
            for kf in range(NQH):
                nc.tensor.matmul(pps[:], otb[:, kf, tb:tb + P],
                                 wo_sb[:, kf, nh * QMW:(nh + 1) * QMW],
                                 start=(kf == 0), stop=(kf == NQH - 1))
            o_t = opool.tile([P, QMW], F32, tag="oo", name=f"ot{t}_{nh}")
            if nh % 2 == 0:
                nc.vector.tensor_copy(o_t[:], pps[:])
            else:
                nc.scalar.copy(o_t[:], pps[:])
            nc.sync.dma_start(
                io["out"][t * P:(t + 1) * P, nh * QMW:(nh + 1) * QMW], o_t[:])

    # ======= software-pipelined schedule =======
    # Block j's qkv/norm work is interleaved (in each engine's static order)
    # with block j-1's attention + o_proj so the PE never waits on the serial
    # ACT->DVE norm chain.
    for t0 in range(2, 4):
        prefetch_x(t0)
    qTbs, otbs = {}, {}
    LAG = 1   # attention for block j runs while qkv of block j+LAG executes
    for slot in range(QM + LAG):
        j = slot            # qkv block index
        ja = slot - LAG     # attention/oproj block index
        if j == 1:
            for kf in range(NQH):
                nc.sync.dma_start(wo_sb[:, kf, :],
                                  io["wot"][kf * P:(kf + 1) * P, :])
        if j < QM:
            qTbs[j] = blk.tile([P, NQH, QMW], F32R, tag="qtb", name=f"qTb{j}")
            otbs[j] = blko.tile([P, NQH, QMW], F32R, tag="otb", name=f"otb{j}")
        for step in range(4):
            t = 4 * j + step
            if j < QM:
                qkv_block(t)
            if ja >= 0:
                attn_block(step, ja, qTbs[ja], otbs[ja])
            if j < QM:
                tp_block(t, qTbs[j])
        if ja >= 0:
            for t2 in range(4 * ja, 4 * ja + 4):
                oproj_block(t2, otbs[ja])
            del qTbs[ja], otbs[ja]


_cache = {}


def _build(apply_qw, apply_kw):
    key = (apply_qw, apply_kw)
    if key in _cache:
        return _cache[key]
    nc = bacc.Bacc("TRN2", target_bir_lowering=False, debug=False)
    io = {
        "xt": nc.dram_tensor("xt", (P, TOK_T, KT, P), F32R, kind="ExternalInput")[:],
        "wt": nc.dram_tensor("wt", (HID, 512 + 2 * P), F32R, kind="ExternalInput")[:],
        "wot": nc.dram_tensor("wot", (NQH * P, HID), F32R, kind="ExternalInput")[:],
        "cos": nc.dram_tensor("cos", (P, TOK_T, D // 2), F32, kind="ExternalInput")[:],
        "sin": nc.dram_tensor("sin", (P, TOK_T, D // 2), F32, kind="ExternalInput")[:],
        "masks": nc.dram_tensor("masks", (P, NQH, QMW), F32R, kind="ExternalInput")[:],
        "ident": nc.dram_tensor("ident", (P, P), F32R, kind="ExternalInput")[:],
        "ones": nc.dram_tensor("ones", (P, P), F32R, kind="ExternalInput")[:],
        "out": nc.dram_tensor("out", (Q, HID), F32, kind="ExternalOutput")[:],
    }
    if apply_qw:
        io["wqrep"] = nc.dram_tensor("wqrep", (P, NQH * P), F32,
                                     kind="ExternalInput")[:]
    if apply_kw:
        io["wkrep"] = nc.dram_tensor("wkrep", (P, P), F32,
                                     kind="ExternalInput")[:]
    with tile.TileContext(nc) as tc:
        with ExitStack() as ctx:
            _emit(ctx, tc, io, apply_qw, apply_kw)
    nc.compile()
    _cache[key] = nc
    return nc


def kernel(positions, hidden_states, k_cache, v_cache, wqkv, wo, q_norm_w,
           k_norm_w, seq_len):
    global last_exec_time_ns
    positions = np.asarray(positions)
    hidden_states = np.asarray(hidden_states, dtype=np.float32)
    wqkv = np.asarray(wqkv, dtype=np.float32)
    wo = np.asarray(wo, dtype=np.float32)
    q_norm_w = np.asarray(q_norm_w, dtype=np.float32)
    k_norm_w = np.asarray(k_norm_w, dtype=np.float32)
    if int(np.asarray(seq_len)) != Q:
        raise NotImplementedError("kernel compiled for seq_len == qlen == 2048")

    apply_qw = not np.all(q_norm_w == 1.0)
    apply_kw = not np.all(k_norm_w == 1.0)
    nc = _build(apply_qw, apply_kw)

    # rope tables per batch (mirrors reference fp32 arithmetic)
    inv_freq = 1.0 / (np.float32(THETA) **
                      (np.arange(0, D, 2, dtype=np.float32) / np.float32(D)))
    # causal mask tiles for the 4 diagonal offsets
    p_idx = np.arange(P, dtype=np.int64)[:, None]
    f_idx = np.arange(QMW, dtype=np.int64)[None, :]
    masks = np.zeros((NQH, P, QMW), dtype=np.float32)
    for r in range(NQH):
        masks[r] = np.where(f_idx >= p_idx + r * P, 0.0, MASK_NEG)
    masksr = np.ascontiguousarray(masks.transpose(1, 0, 2))  # [p, r, f]
    ident = np.eye(P, dtype=np.float32)
    ones = np.ones((P, P), dtype=np.float32)

    in_maps = []
    for c in range(8):
        b, g = c // 4, c % 4
        # pre-tiled x^T: xr[p, t, kt, m] = hidden[b][t*128+m, kt*128+p]
        xt = np.ascontiguousarray(
            hidden_states[b].T.reshape(KT, P, TOK_T, P).transpose(1, 2, 0, 3))
        wq = wqkv[512 * g:512 * (g + 1)]
        wk = wqkv[HQ * D + P * g: HQ * D + P * (g + 1)]
        wv = wqkv[HQ * D + HKV * D + P * g: HQ * D + HKV * D + P * (g + 1)]
        wt = np.ascontiguousarray(np.concatenate([wq, wk, wv], axis=0).T)
        wot = np.ascontiguousarray(wo[:, 512 * g:512 * (g + 1)].T)
        freqs = positions[b].astype(np.float32)[:, None] * inv_freq[None, :]
        cosf = np.cos(freqs).astype(np.float32)
        sinf = np.sin(freqs).astype(np.float32)
        # pre-tiled [p, t, d] layouts for 4KB DMA descriptors
        cosr = np.ascontiguousarray(
            cosf.reshape(TOK_T, P, D // 2).transpose(1, 0, 2))
        sinr = np.ascontiguousarray(
            sinf.reshape(TOK_T, P, D // 2).transpose(1, 0, 2))
        m = {
            "xt": xt, "wt": wt, "wot": wot,
            "cos": cosr, "sin": sinr,
            "masks": masksr, "ident": ident, "ones": ones,
        }
        if apply_qw:
            m["wqrep"] = np.broadcast_to(
                np.tile(q_norm_w, NQH)[None, :], (P, NQH * P)).copy()
        if apply_kw:
            m["wkrep"] = np.broadcast_to(k_norm_w[None, :], (P, P)).copy()
        in_maps.append(m)

    trace = bool(os.environ.get("BASS_TRACE"))
    res = run_bass_kernel_spmd(nc, in_maps, core_ids=list(range(8)),
                               trace=trace)
    last_exec_time_ns = res.exec_time_ns

    out = np.empty((B, Q, HID), dtype=np.float32)
    for b in range(B):
        acc = res.results[4 * b]["out"].astype(np.float32).copy()
        for g in range(1, 4):
            acc += res.results[4 * b + g]["out"]
        out[b] = acc
    return out


# revision 26
# speedup vs baseline: 1.1090x; 1.0165x over previous
"""Trainium2 Bass kernel for CodePredictorAttention (B=2, Q=2048, HID=2048,
HQ=16, HKV=4, D=128, causal, qk-rmsnorm + neox rope, GQA).

Sharding (8 cores): data-parallel over batch (2) x tensor-parallel over head
groups (4). Core c handles batch c//4 and q-heads [4g, 4g+4) with kv-head g,
g = c%4. o_proj is row-parallel; the 4 partial outputs per batch are summed
on the host.

Per-core pipeline (all matmuls in float32r: full PE speed, ~12-bit mantissa):
  1. qkv projection  out[tok, feat] = x^T-tiles.T @ w-tiles   (feat = 4q+k+v)
  2. rms-norm scale via DVE (sumsq + rsqrt Newton), applied during PSUM
     eviction (ACT copy with per-partition scale); neox rope on DVE;
     q/k transposed to [D, tok] via PE transposes.
  3. attention in S^T layout: S^T[k,q] = kT.T @ qT (+ causal mask tiles via
     identity matmul), E = exp(S^T * scale) on ACT, O^T[D,q] = V.T @ E and
     colsums = ones.T @ E accumulated on PE; normalize O^T = O^T * (1/sums)
     on DVE.
  4. o_proj out[tok, hid] = O^T-tiles.T @ wo^T-tiles, DMA to DRAM.
"""
import os
import numpy as np
from contextlib import ExitStack

import concourse.bass as bass
import concourse.tile as tile
from concourse import bacc, mybir
from concourse.bass_utils import run_bass_kernel_spmd

B, Q, HID = 2, 2048, 2048
HQ, HKV, D = 16, 4, 128
NQH = HQ // HKV          # q heads per core = 4
EPS = 1e-6
THETA = 1000000.0
SCALE = float(D) ** -0.5
MASK_NEG = -30000.0
P = 128
TOK_T = Q // P           # 16 token tiles
KT = HID // P            # 16 hid contraction tiles
QM = 4                   # q-macro tiles of 512
QMW = Q // QM            # 512
F32 = mybir.dt.float32
F32R = mybir.dt.float32r
I32 = mybir.dt.int32
AF = mybir.ActivationFunctionType
OP = mybir.AluOpType

RSQRT_MAGIC = 0x5F3759DF

last_exec_time_ns = None   # set when BASS_TRACE=1


def _emit(ctx, tc, io, apply_qw, apply_kw):
    nc = tc.nc

    const = ctx.enter_context(tc.tile_pool(name="const", bufs=1))
    xpool = ctx.enter_context(tc.tile_pool(name="xp", bufs=3))
    qkvsb = ctx.enter_context(tc.tile_pool(name="qkvsb", bufs=2))
    rsq = ctx.enter_context(tc.tile_pool(name="rsq", bufs=5))
    big = ctx.enter_context(tc.tile_pool(name="big", bufs=1))
    blk = ctx.enter_context(tc.tile_pool(name="blk", bufs=2))
    blko = ctx.enter_context(tc.tile_pool(name="blko", bufs=2))
    epool = ctx.enter_context(tc.tile_pool(name="ep", bufs=6))
    opool = ctx.enter_context(tc.tile_pool(name="op", bufs=2))
    recp = ctx.enter_context(tc.tile_pool(name="recp", bufs=2))
    scrp = ctx.enter_context(tc.tile_pool(name="scrp", bufs=2))
    psum = ctx.enter_context(tc.tile_pool(name="ps", bufs=6, space="PSUM"))
    psum_kv = ctx.enter_context(tc.tile_pool(name="pskv", bufs=2, space="PSUM"))

    # ---- earliest x tiles first: the very first matmuls need them ----
    early_x = {}
    for t0 in range(2):
        ex = xpool.tile([P, KT, P], F32R, tag="x", name=f"x{t0}")
        for kc in range(0, KT, 8):
            nc.sync.dma_start(ex[:, kc:kc + 8, :], io["xt"][:, t0, kc:kc + 8, :])
        early_x[t0] = ex

    # ---- resident constants / weights ----
    w_sb = const.tile([P, KT, 512 + 2 * P], F32R, tag="wbig")  # qkv w [p, kt, f]
    # early k-slices split in halves across queues; first matmuls start early
    FW = 512 + 2 * P
    for k in range(KT):
        if k < 4:
            nc.sync.dma_start(w_sb[:, k, 0:FW // 2],
                              io["wt"][k * P:(k + 1) * P, 0:FW // 2])
            nc.sync.dma_start(w_sb[:, k, FW // 2:],
                              io["wt"][k * P:(k + 1) * P, FW // 2:])
        else:
            nc.sync.dma_start(w_sb[:, k, :],
                              io["wt"][k * P:(k + 1) * P, :])
    cos_sb = const.tile([P, TOK_T, D // 2], F32)
    nc.sync.dma_start(cos_sb[:], io["cos"][:])
    sin_sb = const.tile([P, TOK_T, D // 2], F32)
    nc.sync.dma_start(sin_sb[:], io["sin"][:])
    mask_sb = const.tile([P, NQH, QMW], F32R)
    nc.sync.dma_start(mask_sb[:], io["masks"][:])
    ident_sb = const.tile([P, P], F32R)
    nc.sync.dma_start(ident_sb[:], io["ident"][:])
    ones_sb = const.tile([P, P], F32R)
    nc.sync.dma_start(ones_sb[:], io["ones"][:])
    if apply_qw:
        wqrep_sb = const.tile([P, NQH * P], F32)
        nc.sync.dma_start(wqrep_sb[:], io["wqrep"][:])
    if apply_kw:
        wkrep_sb = const.tile([P, P], F32)
        nc.sync.dma_start(wkrep_sb[:], io["wkrep"][:])
    magic_sb = const.tile([P, NQH + 1], I32)
    nc.vector.memset(magic_sb[:], RSQRT_MAGIC)

    # ---- resident activations ----
    kT_sb = big.tile([P, Q], F32R)           # [D, tok]
    v_sb = big.tile([P, TOK_T, D], F32R)     # [tok%128, t, D]

    def rsqrt_dve(out_ap, in_ap, n):
        """out = in^-1/2 elementwise on DVE: bit-trick seed + 2 Newton steps."""
        y = rsq.tile([P, n], F32, tag="rs_y")
        sh = rsq.tile([P, n], I32, tag="rs_sh")
        nc.vector.tensor_scalar(sh[:], in_ap.bitcast(I32), 1, None,
                                op0=OP.arith_shift_right)
        nc.vector.tensor_sub(y[:].bitcast(I32), magic_sb[:, :n], sh[:])
        for it in range(2):
            a = rsq.tile([P, n], F32, tag="rs_a")
            c = rsq.tile([P, n], F32, tag="rs_c")
            nc.vector.tensor_mul(a[:], y[:], y[:])
            nc.vector.tensor_mul(a[:], a[:], in_ap)
            nc.vector.tensor_scalar(c[:], a[:], -0.5, 1.5, op0=OP.mult, op1=OP.add)
            if it == 0:
                yn = rsq.tile([P, n], F32, tag="rs_y2")
                nc.vector.tensor_mul(yn[:], y[:], c[:])
                y = yn
            else:
                nc.vector.tensor_mul(out_ap, y[:], c[:])

    # o_proj weights resident alongside qkv weights; DMA emitted later (at
    # the start of block j=1) so it does not crowd startup bandwidth
    wo_sb = const.tile([P, NQH, HID], F32R)

    NH = NQH + 1
    d2 = D // 2
    sq_scale = float(D) ** -0.5

    qkn_tiles = {}
    x_tiles = dict(early_x)

    def prefetch_x(t):
        x_sb = xpool.tile([P, KT, P], F32R, tag="x", name=f"x{t}")
        for kc in range(0, KT, 8):
            nc.sync.dma_start(x_sb[:, kc:kc + 8, :],
                              io["xt"][:, t, kc:kc + 8, :])
        x_tiles[t] = x_sb

    def qkv_block(t):
        """qkv matmuls + rmsnorm + rope for token tile t (qk_n stashed for
        the separately-emitted tp_block)."""
        if t not in x_tiles:
            prefetch_x(t)
        x_sb = x_tiles.pop(t)
        qps = psum.tile([P, NQH * P], F32, tag="a", name=f"qps{t}")
        kvps = psum_kv.tile([P, 2 * P], F32, tag="kv", name=f"kvps{t}")
        for k in range(KT):
            nc.tensor.matmul(qps[:], x_sb[:, k, :], w_sb[:, k, 0:NQH * P],
                             start=(k == 0), stop=(k == KT - 1))
        for k in range(KT):
            nc.tensor.matmul(kvps[:], x_sb[:, k, :], w_sb[:, k, NQH * P:],
                             start=(k == 0), stop=(k == KT - 1))

        # evict PSUM fast; k stacked behind the 4 q heads (5 lanes)
        qk = qkvsb.tile([P, NH * P], F32, tag="qk", name=f"qk{t}")
        nc.scalar.copy(v_sb[:, t, :], kvps[:, P:2 * P])
        nc.scalar.copy(qk[:, NQH * P:], kvps[:, 0:P])
        nc.scalar.copy(qk[:, 0:NQH * P], qps[:])

        # mean-square per lane: ACT Square (in every table set) + accum_out
        msq = rsq.tile([P, NH], F32, tag="msq", name=f"msq{t}")
        for h in range(NH):
            scr = scrp.tile([P, P], F32, tag="scr", name=f"scr{t}_{h}")
            nc.scalar.activation(scr[:], qk[:, h * P:(h + 1) * P], AF.Square,
                                 scale=sq_scale, accum_out=msq[:, h:h + 1])
        msqe = rsq.tile([P, NH], F32, tag="msqe", name=f"msqe{t}")
        nc.vector.tensor_scalar(msqe[:], msq[:], EPS, None, op0=OP.add)
        rstd = rsq.tile([P, NH], F32, tag="rstd", name=f"rstd{t}")
        rsqrt_dve(rstd[:], msqe[:], NH)

        # per-lane rstd scale in place on DVE
        for h in range(NH):
            nc.vector.tensor_scalar(qk[:, h * P:(h + 1) * P],
                                    qk[:, h * P:(h + 1) * P],
                                    rstd[:, h:h + 1], None, op0=OP.mult)
        if apply_qw:
            nc.vector.tensor_mul(qk[:, 0:NQH * P], qk[:, 0:NQH * P],
                                 wqrep_sb[:])
        if apply_kw:
            nc.vector.tensor_mul(qk[:, NQH * P:], qk[:, NQH * P:],
                                 wkrep_sb[:])

        # neox rope fused across the 5 lanes; subtract/add reuse qk_n in place
        cosb = cos_sb[:, t:t + 1, :].to_broadcast([P, NH, d2])
        sinb = sin_sb[:, t:t + 1, :].to_broadcast([P, NH, d2])
        qv = qk[:].rearrange("p (h d) -> p h d", h=NH)
        qk_n = qkvsb.tile([P, NH * P], F32R, tag="qkn", name=f"qkn{t}")
        qnv = qk_n[:].rearrange("p (h d) -> p h d", h=NH)
        t1 = qkvsb.tile([P, NH * d2], F32, tag="t1", name=f"t1_{t}")
        t1v = t1[:].rearrange("p (h d) -> p h d", h=NH)
        nc.vector.tensor_mul(qnv[:, :, 0:d2], qv[:, :, d2:D], sinb)
        nc.vector.tensor_mul(t1v, qv[:, :, 0:d2], cosb)
        nc.vector.tensor_sub(qnv[:, :, 0:d2], t1v, qnv[:, :, 0:d2])
        nc.vector.tensor_mul(qnv[:, :, d2:D], qv[:, :, 0:d2], sinb)
        nc.vector.tensor_mul(t1v, qv[:, :, d2:D], cosb)
        nc.vector.tensor_add(qnv[:, :, d2:D], t1v, qnv[:, :, d2:D])
        qkn_tiles[t] = qk_n

    def tp_block(t, qTb):
        """transpose the 5 rope'd lanes into [D, tok] stores (DVE evicts)"""
        tb = (t % 4) * P
        qk_n = qkn_tiles.pop(t)
        for h in range(NH):
            tp = psum.tile([P, P], F32R, tag="a", name=f"tp{t}_{h}")
            nc.tensor.transpose(tp[:], qk_n[:, h * P:(h + 1) * P], ident_sb[:])
            if h < NQH:
                nc.vector.tensor_copy(qTb[:, h, tb:tb + P], tp[:])
            else:
                nc.vector.tensor_copy(kT_sb[:, t * P:(t + 1) * P], tp[:])

    def attn_block(h, j, qTb, otb):
        """causal attention for head h, q-macro j (S^T layout), software-
        pipelined so S(i+1) runs on PE while ACT computes exp(i)."""
        nk = 4 * j + 4
        ops_ = psum.tile([P, QMW], F32, tag="a", name=f"ops{h}_{j}")
        sums = psum_kv.tile([P, QMW], F32, tag="kv", name=f"sums{h}_{j}")

        def s_off(i):
            # diagonal tile at offset r: columns < 128*r are fully masked
            return max(0, (i - 4 * j)) * P

        def s_mm(i):
            diag = i >= 4 * j
            off = s_off(i)
            sps = psum.tile([P, QMW], F32, tag="a", name=f"sps{h}_{j}_{i}")
            nc.tensor.matmul(sps[:, off:], kT_sb[:, i * P:(i + 1) * P],
                             qTb[:, h, off:], start=True, stop=not diag)
            if diag:
                nc.tensor.matmul(sps[:, off:], ident_sb[:],
                                 mask_sb[:, i - 4 * j, off:],
                                 start=False, stop=True)
            return sps

        sps = s_mm(0)
        for i in range(nk):
            off = s_off(i)
            e = epool.tile([P, QMW], F32R, tag="e", name=f"e{h}_{j}_{i}")
            nc.scalar.activation(e[:, off:], sps[:, off:], AF.Exp, scale=SCALE)
            if i + 1 < nk:
                sps = s_mm(i + 1)
            nc.tensor.matmul(ops_[:, off:], v_sb[:, i, :], e[:, off:],
                             start=(i == 0), stop=(i == nk - 1))
            nc.tensor.matmul(sums[:, off:], ones_sb[:], e[:, off:],
                             start=(i == 0), stop=(i == nk - 1))
        rec = recp.tile([P, QMW], F32, tag="rec", name=f"rec{h}_{j}")
        nc.vector.reciprocal_approx_fast(out=rec[:], in_=sums[:])
        nc.vector.tensor_mul(otb[:, h, :], ops_[:], rec[:])

    def oproj_block(t, otb):
        tb = (t % 4) * P
        for nh in range(NQH):
            pps = psum.tile([P, QMW], F32, tag="a", name=f"pps{t}_{nh}")
            for kf in range(NQH):
                nc.tensor.matmul(pps[:], otb[:, kf, tb:tb + P],
                                 wo_sb[:, kf, nh * QMW:(nh + 1) * QMW],
                                 start=(kf == 0), stop=(kf == NQH - 1))
            o_t = opool.tile([P, QMW], F32, tag="oo", name=f"ot{t}_{nh}")
            if nh % 2 == 0:
                nc.vector.tensor_copy(o_t[:], pps[:])
            else:
                nc.scalar.copy(o_t[:], pps[:])
            nc.sync.dma_start(
                io["out"][t * P:(t + 1) * P, nh * QMW:(nh + 1) * QMW], o_t[:])

    # ======= software-pipelined schedule =======
    # Block j's qkv/norm work is interleaved (in each engine's static order)
    # with block j-1's attention + o_proj so the PE never waits on the serial
    # ACT->DVE norm chain.
    for t0 in range(2, 4):
        prefetch_x(t0)
    qTbs, otbs = {}, {}
    LAG = 1   # attention for block j runs while qkv of block j+LAG executes
    for slot in range(QM + LAG):
        j = slot            # qkv block index
        ja = slot - LAG     # attention/oproj block index
        if j == 1:
            for kf in range(NQH):
                nc.sync.dma_start(wo_sb[:, kf, :],
                                  io["wot"][kf * P:(kf + 1) * P, :])
        if j < QM:
            qTbs[j] = blk.tile([P, NQH, QMW], F32R, tag="qtb", name=f"qTb{j}")
            otbs[j] = blko.tile([P, NQH, QMW], F32R, tag="otb", name=f"otb{j}")
        for step in range(4):
            t = 4 * j + step
            if j < QM:
                qkv_block(t)
            if ja >= 0:
                attn_block(step, ja, qTbs[ja], otbs[ja])
            if j < QM:
                tp_block(t, qTbs[j])
        if ja >= 0:
            for t2 in range(4 * ja, 4 * ja + 4):
                oproj_block(t2, otbs[ja])
            del qTbs[ja], otbs[ja]


_cache = {}


def _build(apply_qw, apply_kw):
    key = (apply_qw, apply_kw)
    if key in _cache:
        return _cache[key]
    nc = bacc.Bacc("TRN2", target_bir_lowering=False, debug=False)
    io = {
        "xt": nc.dram_tensor("xt", (P, TOK_T, KT, P), F32R, kind="ExternalInput")[:],
        "wt": nc.dram_tensor("wt", (HID, 512 + 2 * P), F32R, kind="ExternalInput")[:],
        "wot": nc.dram_tensor("wot", (NQH * P, HID), F32R, kind="ExternalInput")[:],
        "cos": nc.dram_tensor("cos", (P, TOK_T, D // 2), F32, kind="ExternalInput")[:],
        "sin": nc.dram_tensor("sin", (P, TOK_T, D // 2), F32, kind="ExternalInput")[:],
        "masks": nc.dram_tensor("masks", (P, NQH, QMW), F32R, kind="ExternalInput")[:],
        "ident": nc.dram_tensor("ident", (P, P), F32R, kind="ExternalInput")[:],
        "ones": nc.dram_tensor("ones", (P, P), F32R, kind="ExternalInput")[:],
        "out": nc.dram_tensor("out", (Q, HID), F32, kind="ExternalOutput")[:],
    }
    if apply_qw:
        io["wqrep"] = nc.dram_tensor("wqrep", (P, NQH * P), F32,
                                     kind="ExternalInput")[:]
    if apply_kw:
        io["wkrep"] = nc.dram_tensor("wkrep", (P, P), F32,
                                     kind="ExternalInput")[:]
    with tile.TileContext(nc) as tc:
        with ExitStack() as ctx:
            _emit(ctx, tc, io, apply_qw, apply_kw)
    nc.compile()
    _cache[key] = nc
    return nc


def kernel(positions, hidden_states, k_cache, v_cache, wqkv, wo, q_norm_w,
           k_norm_w, seq_len):
    global last_exec_time_ns
    positions = np.asarray(positions)
    hidden_states = np.asarray(hidden_states, dtype=np.float32)
    wqkv = np.asarray(wqkv, dtype=np.float32)
    wo = np.asarray(wo, dtype=np.float32)
    q_norm_w = np.asarray(q_norm_w, dtype=np.float32)
    k_norm_w = np.asarray(k_norm_w, dtype=np.float32)
    if int(np.asarray(seq_len)) != Q:
        raise NotImplementedError("kernel compiled for seq_len == qlen == 2048")

    apply_qw = not np.all(q_norm_w == 1.0)
    apply_kw = not np.all(k_norm_w == 1.0)
    nc = _build(apply_qw, apply_kw)

    # rope tables per batch (mirrors reference fp32 arithmetic)
    inv_freq = 1.0 / (np.float32(THETA) **
                      (np.arange(0, D, 2, dtype=np.float32) / np.float32(D)))
    # causal mask tiles for the 4 diagonal offsets
    p_idx = np.arange(P, dtype=np.int64)[:, None]
    f_idx = np.arange(QMW, dtype=np.int64)[None, :]
    masks = np.zeros((NQH, P, QMW), dtype=np.float32)
    for r in range(NQH):
        masks[r] = np.where(f_idx >= p_idx + r * P, 0.0, MASK_NEG)
    masksr = np.ascontiguousarray(masks.transpose(1, 0, 2))  # [p, r, f]
    ident = np.eye(P, dtype=np.float32)
    ones = np.ones((P, P), dtype=np.float32)

    in_maps = []
    for c in range(8):
        b, g = c // 4, c % 4
        # pre-tiled x^T: xr[p, t, kt, m] = hidden[b][t*128+m, kt*128+p]
        xt = np.ascontiguousarray(
            hidden_states[b].T.reshape(KT, P, TOK_T, P).transpose(1, 2, 0, 3))
        wq = wqkv[512 * g:512 * (g + 1)]
        wk = wqkv[HQ * D + P * g: HQ * D + P * (g + 1)]
        wv = wqkv[HQ * D + HKV * D + P * g: HQ * D + HKV * D + P * (g + 1)]
        wt = np.ascontiguousarray(np.concatenate([wq, wk, wv], axis=0).T)
        wot = np.ascontiguousarray(wo[:, 512 * g:512 * (g + 1)].T)
        freqs = positions[b].astype(np.float32)[:, None] * inv_freq[None, :]
        cosf = np.cos(freqs).astype(np.float32)
        sinf = np.sin(freqs).astype(np.float32)
        # pre-tiled [p, t, d] layouts for 4KB DMA descriptors
        cosr = np.ascontiguousarray(
            cosf.reshape(TOK_T, P, D // 2).transpose(1, 0, 2))
        sinr = np.ascontiguousarray(
            sinf.reshape(TOK_T, P, D // 2).transpose(1, 0, 2))
        m = {
            "xt": xt, "wt": wt, "wot": wot,
            "cos": cosr, "sin": sinr,
            "masks": masksr, "ident": ident, "ones": ones,
        }
        if apply_qw:
            m["wqrep"] = np.broadcast_to(
                np.tile(q_norm_w, NQH)[None, :], (P, NQH * P)).copy()
        if apply_kw:
            m["wkrep"] = np.broadcast_to(k_norm_w[None, :], (P, P)).copy()
        in_maps.append(m)

    trace = bool(os.environ.get("BASS_TRACE"))
    res = run_bass_kernel_spmd(nc, in_maps, core_ids=list(range(8)),
                               trace=trace)
    last_exec_time_ns = res.exec_time_ns

    out = np.empty((B, Q, HID), dtype=np.float32)
    for b in range(B):
        acc = res.results[4 * b]["out"].astype(np.float32).copy()
        for g in range(1, 4):
            acc += res.results[4 * b + g]["out"]
        out[b] = acc
    return out


# revision 28
# speedup vs baseline: 1.2718x; 1.1468x over previous
"""Trainium2 Bass kernel for CodePredictorAttention (B=2, Q=2048, HID=2048,
HQ=16, HKV=4, D=128, causal, qk-rmsnorm + neox rope, GQA).

Sharding (8 cores): data-parallel over batch (2) x tensor-parallel over head
groups (4). Core c handles batch c//4 and q-heads [4g, 4g+4) with kv-head g,
g = c%4. o_proj is row-parallel; the 4 partial outputs per batch are summed
on the host.

Per-core pipeline (all matmuls in float32r: full PE speed, ~12-bit mantissa):
  1. qkv projection  out[tok, feat] = x^T-tiles.T @ w-tiles   (feat = 4q+k+v)
  2. rms-norm scale via DVE (sumsq + rsqrt Newton), applied during PSUM
     eviction (ACT copy with per-partition scale); neox rope on DVE;
     q/k transposed to [D, tok] via PE transposes.
  3. attention in S^T layout: S^T[k,q] = kT.T @ qT (+ causal mask tiles via
     identity matmul), E = exp(S^T * scale) on ACT, O^T[D,q] = V.T @ E and
     colsums = ones.T @ E accumulated on PE; normalize O^T = O^T * (1/sums)
     on DVE.
  4. o_proj out[tok, hid] = O^T-tiles.T @ wo^T-tiles, DMA to DRAM.
"""
import os
import numpy as np
from contextlib import ExitStack

import concourse.bass as bass
import concourse.tile as tile
from concourse import bacc, mybir
from concourse.bass_utils import run_bass_kernel_spmd

B, Q, HID = 2, 2048, 2048
HQ, HKV, D = 16, 4, 128
NQH = HQ // HKV          # q heads per core = 4
EPS = 1e-6
THETA = 1000000.0
SCALE = float(D) ** -0.5
MASK_NEG = -30000.0
P = 128
TOK_T = Q // P           # 16 token tiles
KT = HID // P            # 16 hid contraction tiles
QM = 4                   # q-macro tiles of 512
QMW = Q // QM            # 512
F32 = mybir.dt.float32
F32R = mybir.dt.float32r
I32 = mybir.dt.int32
AF = mybir.ActivationFunctionType
OP = mybir.AluOpType

RSQRT_MAGIC = 0x5F3759DF

last_exec_time_ns = None   # set when BASS_TRACE=1


def _emit(ctx, tc, io, apply_qw, apply_kw):
    nc = tc.nc

    const = ctx.enter_context(tc.tile_pool(name="const", bufs=1))
    xpool = ctx.enter_context(tc.tile_pool(name="xp", bufs=3))
    qkvsb = ctx.enter_context(tc.tile_pool(name="qkvsb", bufs=2))
    rsq = ctx.enter_context(tc.tile_pool(name="rsq", bufs=5))
    big = ctx.enter_context(tc.tile_pool(name="big", bufs=1))
    blk = ctx.enter_context(tc.tile_pool(name="blk", bufs=2))
    blko = ctx.enter_context(tc.tile_pool(name="blko", bufs=2))
    epool = ctx.enter_context(tc.tile_pool(name="ep", bufs=6))
    opool = ctx.enter_context(tc.tile_pool(name="op", bufs=2))
    recp = ctx.enter_context(tc.tile_pool(name="recp", bufs=2))
    scrp = ctx.enter_context(tc.tile_pool(name="scrp", bufs=2))
    psum = ctx.enter_context(tc.tile_pool(name="ps", bufs=6, space="PSUM"))
    psum_kv = ctx.enter_context(tc.tile_pool(name="pskv", bufs=2, space="PSUM"))

    # ---- earliest x tiles first: the very first matmuls need them ----
    early_x = {}
    for t0 in range(2):
        ex = xpool.tile([P, KT, P], F32R, tag="x", name=f"x{t0}")
        for kc in range(0, KT, 8):
            nc.sync.dma_start(ex[:, kc:kc + 8, :], io["xt"][:, t0, kc:kc + 8, :])
        early_x[t0] = ex

    # ---- resident constants / weights ----
    w_sb = const.tile([P, KT, 512 + 2 * P], F32R, tag="wbig")  # qkv w [p, kt, f]
    # early k-slices split in halves across queues; first matmuls start early
    FW = 512 + 2 * P
    for k in range(KT):
        if k < 4:
            nc.sync.dma_start(w_sb[:, k, 0:FW // 2],
                              io["wt"][k * P:(k + 1) * P, 0:FW // 2])
            nc.sync.dma_start(w_sb[:, k, FW // 2:],
                              io["wt"][k * P:(k + 1) * P, FW // 2:])
        else:
            nc.sync.dma_start(w_sb[:, k, :],
                              io["wt"][k * P:(k + 1) * P, :])
    cos_sb = const.tile([P, TOK_T, D // 2], F32)
    nc.sync.dma_start(cos_sb[:], io["cos"][:])
    sin_sb = const.tile([P, TOK_T, D // 2], F32)
    nc.sync.dma_start(sin_sb[:], io["sin"][:])
    mask_sb = const.tile([P, NQH, QMW], F32R)
    nc.sync.dma_start(mask_sb[:], io["masks"][:])
    ident_sb = const.tile([P, P], F32R)
    nc.sync.dma_start(ident_sb[:], io["ident"][:])
    ones_sb = const.tile([P, P], F32R)
    nc.sync.dma_start(ones_sb[:], io["ones"][:])
    if apply_qw:
        wqrep_sb = const.tile([P, NQH * P], F32)
        nc.sync.dma_start(wqrep_sb[:], io["wqrep"][:])
    if apply_kw:
        wkrep_sb = const.tile([P, P], F32)
        nc.sync.dma_start(wkrep_sb[:], io["wkrep"][:])
    magic_sb = const.tile([P, NQH + 1], I32)
    nc.vector.memset(magic_sb[:], RSQRT_MAGIC)

    # ---- resident activations ----
    kT_sb = big.tile([P, Q], F32R)           # [D, tok]
    v_sb = big.tile([P, TOK_T, D], F32R)     # [tok%128, t, D]

    def rsqrt_dve(out_ap, in_ap, n):
        """out = in^-1/2 elementwise on DVE: bit-trick seed + 2 Newton steps."""
        y = rsq.tile([P, n], F32, tag="rs_y")
        sh = rsq.tile([P, n], I32, tag="rs_sh")
        nc.vector.tensor_scalar(sh[:], in_ap.bitcast(I32), 1, None,
                                op0=OP.arith_shift_right)
        nc.vector.tensor_sub(y[:].bitcast(I32), magic_sb[:, :n], sh[:])
        for it in range(2):
            a = rsq.tile([P, n], F32, tag="rs_a")
            c = rsq.tile([P, n], F32, tag="rs_c")
            nc.vector.tensor_mul(a[:], y[:], y[:])
            nc.vector.tensor_mul(a[:], a[:], in_ap)
            nc.vector.tensor_scalar(c[:], a[:], -0.5, 1.5, op0=OP.mult, op1=OP.add)
            if it == 0:
                yn = rsq.tile([P, n], F32, tag="rs_y2")
                nc.vector.tensor_mul(yn[:], y[:], c[:])
                y = yn
            else:
                nc.vector.tensor_mul(out_ap, y[:], c[:])

    # o_proj weights resident alongside qkv weights; DMA emitted later (at
    # the start of block j=1) so it does not crowd startup bandwidth
    wo_sb = const.tile([P, NQH, HID], F32R)

    NH = NQH + 1
    d2 = D // 2
    sq_scale = float(D) ** -0.5

    qkn_tiles = {}
    x_tiles = dict(early_x)

    def prefetch_x(t):
        x_sb = xpool.tile([P, KT, P], F32R, tag="x", name=f"x{t}")
        for kc in range(0, KT, 8):
            nc.sync.dma_start(x_sb[:, kc:kc + 8, :],
                              io["xt"][:, t, kc:kc + 8, :])
        x_tiles[t] = x_sb

    def qkv_block(t):
        """qkv matmuls + rmsnorm + rope for token tile t (qk_n stashed for
        the separately-emitted tp_block)."""
        if t not in x_tiles:
            prefetch_x(t)
        x_sb = x_tiles.pop(t)
        qps = psum.tile([P, NQH * P], F32, tag="a", name=f"qps{t}")
        kvps = psum_kv.tile([P, 2 * P], F32, tag="kv", name=f"kvps{t}")
        for k in range(KT):
            nc.tensor.matmul(qps[:], x_sb[:, k, :], w_sb[:, k, 0:NQH * P],
                             start=(k == 0), stop=(k == KT - 1))
        for k in range(KT):
            nc.tensor.matmul(kvps[:], x_sb[:, k, :], w_sb[:, k, NQH * P:],
                             start=(k == 0), stop=(k == KT - 1))

        # evict PSUM fast; k stacked behind the 4 q heads (5 lanes)
        qk = qkvsb.tile([P, NH * P], F32, tag="qk", name=f"qk{t}")
        nc.scalar.copy(v_sb[:, t, :], kvps[:, P:2 * P])
        nc.scalar.copy(qk[:, NQH * P:], kvps[:, 0:P])
        nc.scalar.copy(qk[:, 0:NQH * P], qps[:])

        # mean-square per lane: ACT Square (in every table set) + accum_out
        msq = rsq.tile([P, NH], F32, tag="msq", name=f"msq{t}")
        for h in range(NH):
            scr = scrp.tile([P, P], F32, tag="scr", name=f"scr{t}_{h}")
            nc.scalar.activation(scr[:], qk[:, h * P:(h + 1) * P], AF.Square,
                                 scale=sq_scale, accum_out=msq[:, h:h + 1])
        msqe = rsq.tile([P, NH], F32, tag="msqe", name=f"msqe{t}")
        nc.vector.tensor_scalar(msqe[:], msq[:], EPS, None, op0=OP.add)
        rstd = rsq.tile([P, NH], F32, tag="rstd", name=f"rstd{t}")
        rsqrt_dve(rstd[:], msqe[:], NH)

        # per-lane rstd scale in place on DVE
        for h in range(NH):
            nc.vector.tensor_scalar(qk[:, h * P:(h + 1) * P],
                                    qk[:, h * P:(h + 1) * P],
                                    rstd[:, h:h + 1], None, op0=OP.mult)
        if apply_qw:
            nc.vector.tensor_mul(qk[:, 0:NQH * P], qk[:, 0:NQH * P],
                                 wqrep_sb[:])
        if apply_kw:
            nc.vector.tensor_mul(qk[:, NQH * P:], qk[:, NQH * P:],
                                 wkrep_sb[:])

        # neox rope fused across the 5 lanes; subtract/add reuse qk_n in place
        cosb = cos_sb[:, t:t + 1, :].to_broadcast([P, NH, d2])
        sinb = sin_sb[:, t:t + 1, :].to_broadcast([P, NH, d2])
        qv = qk[:].rearrange("p (h d) -> p h d", h=NH)
        qk_n = qkvsb.tile([P, NH * P], F32R, tag="qkn", name=f"qkn{t}")
        qnv = qk_n[:].rearrange("p (h d) -> p h d", h=NH)
        t1 = qkvsb.tile([P, NH * d2], F32, tag="t1", name=f"t1_{t}")
        t1v = t1[:].rearrange("p (h d) -> p h d", h=NH)
        nc.vector.tensor_mul(qnv[:, :, 0:d2], qv[:, :, d2:D], sinb)
        nc.vector.tensor_mul(t1v, qv[:, :, 0:d2], cosb)
        nc.vector.tensor_sub(qnv[:, :, 0:d2], t1v, qnv[:, :, 0:d2])
        nc.vector.tensor_mul(qnv[:, :, d2:D], qv[:, :, 0:d2], sinb)
        nc.vector.tensor_mul(t1v, qv[:, :, d2:D], cosb)
        nc.vector.tensor_add(qnv[:, :, d2:D], t1v, qnv[:, :, d2:D])
        qkn_tiles[t] = qk_n

    def tp_block(t, qTb):
        """transpose the 5 rope'd lanes into [D, tok] stores (DVE evicts)"""
        tb = (t % 4) * P
        qk_n = qkn_tiles.pop(t)
        for h in range(NH):
            tp = psum.tile([P, P], F32R, tag="a", name=f"tp{t}_{h}")
            nc.tensor.transpose(tp[:], qk_n[:, h * P:(h + 1) * P], ident_sb[:])
            if h < NQH:
                nc.vector.tensor_copy(qTb[:, h, tb:tb + P], tp[:])
            else:
                nc.vector.tensor_copy(kT_sb[:, t * P:(t + 1) * P], tp[:])

    def attn_block(h, j, qTb, otb):
        """causal attention for head h, q-macro j (S^T layout), software-
        pipelined so S(i+1) runs on PE while ACT computes exp(i)."""
        nk = 4 * j + 4
        ops_ = psum.tile([P, QMW], F32, tag="a", name=f"ops{h}_{j}")
        sums = psum_kv.tile([P, QMW], F32, tag="kv", name=f"sums{h}_{j}")

        def s_off(i):
            # diagonal tile at offset r: columns < 128*r are fully masked
            return max(0, (i - 4 * j)) * P

        def s_mm(i):
            diag = i >= 4 * j
            off = s_off(i)
            sps = psum.tile([P, QMW], F32, tag="a", name=f"sps{h}_{j}_{i}")
            nc.tensor.matmul(sps[:, off:], kT_sb[:, i * P:(i + 1) * P],
                             qTb[:, h, off:], start=True, stop=not diag)
            if diag:
                nc.tensor.matmul(sps[:, off:], ident_sb[:],
                                 mask_sb[:, i - 4 * j, off:],
                                 start=False, stop=True)
            return sps

        sps = s_mm(0)
        for i in range(nk):
            off = s_off(i)
            e = epool.tile([P, QMW], F32R, tag="e", name=f"e{h}_{j}_{i}")
            nc.scalar.activation(e[:, off:], sps[:, off:], AF.Exp, scale=SCALE)
            if i + 1 < nk:
                sps = s_mm(i + 1)
            nc.tensor.matmul(ops_[:, off:], v_sb[:, i, :], e[:, off:],
                             start=(i == 0), stop=(i == nk - 1))
            nc.tensor.matmul(sums[:, off:], ones_sb[:], e[:, off:],
                             start=(i == 0), stop=(i == nk - 1))
        rec = recp.tile([P, QMW], F32, tag="rec", name=f"rec{h}_{j}")
        nc.vector.reciprocal_approx_fast(out=rec[:], in_=sums[:])
        nc.vector.tensor_mul(otb[:, h, :], ops_[:], rec[:])

    def oproj_block(t, otb):
        tb = (t % 4) * P
        for nh in range(NQH):
            pps = psum.tile([P, QMW], F32, tag="a", name=f"pps{t}_{nh}")
            for kf in range(NQH):
                nc.tensor.matmul(pps[:], otb[:, kf, tb:tb + P],
                                 wo_sb[:, kf, nh * QMW:(nh + 1) * QMW],
                                 start=(kf == 0), stop=(kf == NQH - 1))
            o_t = opool.tile([P, QMW], F32, tag="oo", name=f"ot{t}_{nh}")
            if nh % 2 == 0:
                nc.vector.tensor_copy(o_t[:], pps[:])
            else:
                nc.scalar.copy(o_t[:], pps[:])
            nc.sync.dma_start(
                io["out"][t * P:(t + 1) * P, nh * QMW:(nh + 1) * QMW], o_t[:])

    # ======= software-pipelined schedule =======
    # Block j's qkv/norm work is interleaved (in each engine's static order)
    # with block j-1's attention + o_proj so the PE never waits on the serial
    # ACT->DVE norm chain.
    for t0 in range(2, 4):
        prefetch_x(t0)
    qTbs, otbs = {}, {}
    pending_tp = []
    LAG = 1   # attention for block j runs while qkv of block j+LAG executes
    for slot in range(QM + LAG):
        j = slot            # qkv block index
        ja = slot - LAG     # attention/oproj block index
        if j == 1:
            for kf in range(NQH):
                nc.sync.dma_start(wo_sb[:, kf, :],
                                  io["wot"][kf * P:(kf + 1) * P, :])
        if j < QM:
            qTbs[j] = blk.tile([P, NQH, QMW], F32R, tag="qtb", name=f"qTb{j}")
            otbs[j] = blko.tile([P, NQH, QMW], F32R, tag="otb", name=f"otb{j}")
        for step in range(4):
            t = 4 * j + step
            if j < QM:
                qkv_block(t)
            # transposes for the previous tile: one-step delay hides the
            # ACT->DVE norm/rope chain latency behind the next tile's matmuls.
            # Must be emitted before any attention that reads them (Tile deps
            # follow emission order).
            if pending_tp and pending_tp[0][0] < t:
                pt, pb = pending_tp.pop(0)
                tp_block(pt, pb)
            if ja >= 0:
                attn_block(step, ja, qTbs[ja], otbs[ja])
            if j < QM:
                pending_tp.append((t, qTbs[j]))
        if j == QM - 1:
            while pending_tp:
                pt, pb = pending_tp.pop(0)
                tp_block(pt, pb)
        if ja >= 0:
            for t2 in range(4 * ja, 4 * ja + 4):
                oproj_block(t2, otbs[ja])
            del qTbs[ja], otbs[ja]


_cache = {}


def _build(apply_qw, apply_kw):
    key = (apply_qw, apply_kw)
    if key in _cache:
        return _cache[key]
    nc = bacc.Bacc("TRN2", target_bir_lowering=False, debug=False)
    io = {
        "xt": nc.dram_tensor("xt", (P, TOK_T, KT, P), F32R, kind="ExternalInput")[:],
        "wt": nc.dram_tensor("wt", (HID, 512 + 2 * P), F32R, kind="ExternalInput")[:],
        "wot": nc.dram_tensor("wot", (NQH * P, HID), F32R, kind="ExternalInput")[:],
        "cos": nc.dram_tensor("cos", (P, TOK_T, D // 2), F32, kind="ExternalInput")[:],
        "sin": nc.dram_tensor("sin", (P, TOK_T, D // 2), F32, kind="ExternalInput")[:],
        "masks": nc.dram_tensor("masks", (P, NQH, QMW), F32R, kind="ExternalInput")[:],
        "ident": nc.dram_tensor("ident", (P, P), F32R, kind="ExternalInput")[:],
        "ones": nc.dram_tensor("ones", (P, P), F32R, kind="ExternalInput")[:],
        "out": nc.dram_tensor("out", (Q, HID), F32, kind="ExternalOutput")[:],
    }
    if apply_qw:
        io["wqrep"] = nc.dram_tensor("wqrep", (P, NQH * P), F32,
                                     kind="ExternalInput")[:]
    if apply_kw:
        io["wkrep"] = nc.dram_tensor("wkrep", (P, P), F32,
                                     kind="ExternalInput")[:]
    with tile.TileContext(nc) as tc:
        with ExitStack() as ctx:
            _emit(ctx, tc, io, apply_qw, apply_kw)
    nc.compile()
    _cache[key] = nc
    return nc


def kernel(positions, hidden_states, k_cache, v_cache, wqkv, wo, q_norm_w,
           k_norm_w, seq_len):
    global last_exec_time_ns
    positions = np.asarray(positions)
    hidden_states = np.asarray(hidden_states, dtype=np.float32)
    wqkv = np.asarray(wqkv, dtype=np.float32)
    wo = np.asarray(wo, dtype=np.float32)
    q_norm_w = np.asarray(q_norm_w, dtype=np.float32)
    k_norm_w = np.asarray(k_norm_w, dtype=np.float32)
    if int(np.asarray(seq_len)) != Q:
        raise NotImplementedError("kernel compiled for seq_len == qlen == 2048")

    apply_qw = not np.all(q_norm_w == 1.0)
    apply_kw = not np.all(k_norm_w == 1.0)
    nc = _build(apply_qw, apply_kw)

    # rope tables per batch (mirrors reference fp32 arithmetic)
    inv_freq = 1.0 / (np.float32(THETA) **
                      (np.arange(0, D, 2, dtype=np.float32) / np.float32(D)))
    # causal mask tiles for the 4 diagonal offsets
    p_idx = np.arange(P, dtype=np.int64)[:, None]
    f_idx = np.arange(QMW, dtype=np.int64)[None, :]
    masks = np.zeros((NQH, P, QMW), dtype=np.float32)
    for r in range(NQH):
        masks[r] = np.where(f_idx >= p_idx + r * P, 0.0, MASK_NEG)
    masksr = np.ascontiguousarray(masks.transpose(1, 0, 2))  # [p, r, f]
    ident = np.eye(P, dtype=np.float32)
    ones = np.ones((P, P), dtype=np.float32)

    in_maps = []
    for c in range(8):
        b, g = c // 4, c % 4
        # pre-tiled x^T: xr[p, t, kt, m] = hidden[b][t*128+m, kt*128+p]
        xt = np.ascontiguousarray(
            hidden_states[b].T.reshape(KT, P, TOK_T, P).transpose(1, 2, 0, 3))
        wq = wqkv[512 * g:512 * (g + 1)]
        wk = wqkv[HQ * D + P * g: HQ * D + P * (g + 1)]
        wv = wqkv[HQ * D + HKV * D + P * g: HQ * D + HKV * D + P * (g + 1)]
        wt = np.ascontiguousarray(np.concatenate([wq, wk, wv], axis=0).T)
        wot = np.ascontiguousarray(wo[:, 512 * g:512 * (g + 1)].T)
        freqs = positions[b].astype(np.float32)[:, None] * inv_freq[None, :]
        cosf = np.cos(freqs).astype(np.float32)
        sinf = np.sin(freqs).astype(np.float32)
        # pre-tiled [p, t, d] layouts for 4KB DMA descriptors
        cosr = np.ascontiguousarray(
            cosf.reshape(TOK_T, P, D // 2).transpose(1, 0, 2))
        sinr = np.ascontiguousarray(
            sinf.reshape(TOK_T, P, D // 2).transpose(1, 0, 2))
        m = {
            "xt": xt, "wt": wt, "wot": wot,
            "cos": cosr, "sin": sinr,
            "masks": masksr, "ident": ident, "ones": ones,
        }
        if apply_qw:
            m["wqrep"] = np.broadcast_to(
                np.tile(q_norm_w, NQH)[None, :], (P, NQH * P)).copy()
        if apply_kw:
            m["wkrep"] = np.broadcast_to(k_norm_w[None, :], (P, P)).copy()
        in_maps.append(m)

    trace = bool(os.environ.get("BASS_TRACE"))
    res = run_bass_kernel_spmd(nc, in_maps, core_ids=list(range(8)),
                               trace=trace)
    last_exec_time_ns = res.exec_time_ns

    out = np.empty((B, Q, HID), dtype=np.float32)
    for b in range(B):
        acc = res.results[4 * b]["out"].astype(np.float32).copy()
        for g in range(1, 4):
            acc += res.results[4 * b + g]["out"]
        out[b] = acc
    return out
